# revision 1
# baseline (speedup 1.0000x reference)
"""MiMo V2 MoE gate (sigmoid routing, grouped top-k) on 8 Trainium2 cores.

Contract: kernel(**inputs) takes the FULL unsharded inputs
(hidden_states [4,4096,4096] f32, weight [256,4096] f32,
e_score_correction_bias [256] f32) and returns (topk_idx int32 [16384,8],
topk_weight f32 [16384,8]) matching reference.py.

Strategy (data-parallel over tokens):
  - 16384 tokens are sharded 2048/core across 8 NeuronCores.
  - Host pre-transposes each x shard to [4096, 2048] so the device can
    feed the PE's contraction (partition) dim directly; weight.T [4096,256]
    and the bias (pre-broadcast to [128,256]) are replicated.
  - Per core: gate GEMM in fp32 (PSUM accumulation over 32 k-chunks),
    sigmoid on ScalarE, then the grouped top-k entirely on VectorE using
    the DVE sort8 primitives (max / max_index / match_replace). The
    topk weights (scores at the selected experts, ordered by biased
    score rank) are recovered without any gather via an 8x8 index-match
    between the two sort orders.
"""

import sys

if "/opt/trn_rl_repo" not in sys.path:
    sys.path.insert(0, "/opt/trn_rl_repo")

import numpy as np

import concourse.bass as bass
import concourse.mybir as mybir
import concourse.tile as tile
from concourse.tile_rust import add_dep_helper, annotate_deps

P = 128
H = 4096
E = 256
N_CORES = 8
T_FULL = 16384
T_CORE = T_FULL // N_CORES  # 2048
KC = H // P                 # 32 contraction chunks
TOK_TILES = T_CORE // P     # 16 token tiles per core
N_GROUP = 8
EG = E // N_GROUP           # 32 experts per group
TOPK_GROUP = 4
TOP_K = 8
ROUTED_SCALING = 2.5
NEG = -1e30

F32 = mybir.dt.float32
U32 = mybir.dt.uint32
AF = mybir.ActivationFunctionType
OP = mybir.AluOpType

# dtype used for the matmul operand tiles (float32 = exact 4-pass;
# float32r = relaxed-precision full-speed variant)
MM_DT = F32


def _reserve(nc, eng, X, n, prev=None):
    """Emit n plain nops on X's engine, ordered after `prev` (a
    BassInstruction or None) and before X. They act as spare 1-wait
    carriers for _legalize_waits (every TPB instruction has exactly one
    HW wait slot; Tile can assign several waits to one instruction,
    which walrus then rejects)."""
    last = prev.ins if prev is not None else None
    for _ in range(n):
        nop = eng.nop(nofuse=True)
        if last is not None:
            add_dep_helper(nop.ins, last, sync=False,
                           reason="chain reserve nop after predecessor")
        add_dep_helper(X.ins, nop.ins, sync=False,
                       reason="reserve nop precedes its instruction")
        last = nop.ins


def _legalize_waits(nc, report=None):
    """Every TPB instruction has ONE hardware wait slot; Tile can assign
    several on_wait entries to an instruction, which walrus rejects
    ("Too many sync wait commands"). Fix in two ways, per engine stream
    (scheduled order):
      1. value-floor dedup: drop waits already implied by an earlier wait
         on the same semaphore in this stream (monotonic sems).
      2. excess-wait hoisting: move extra waits onto the nearest earlier
         wait-free instruction, scanning only across instructions with no
         on_update (pure nops) -- crossing an updater could reorder a
         producer chain and deadlock; this rule keeps placements provably
         safe. _reserve() plants such nops next to risky instructions.
    Drains are skipped (they encode multi-sem waits natively)."""
    stop_types = (
        mybir.InstDrain,
        mybir.InstEventSemaphore,
        mybir.InstCall,
    )
    leftover = []
    if True:
        # The kernel CFG is linear (main block -> end block), so per-engine
        # program order is the block-order concatenation. Crossing an
        # unconditional branch just means waiting before the jump.
        streams = {}
        nonmono = set()  # sems that ever decrease (barrier sems): no
                         # floor-dedup and no relocation for their waits
        for blk in nc.m.functions[0].blocks:
            for inst in blk.instructions:
                streams.setdefault(str(inst.engine), []).append(inst)
                si = inst.sync_info
                for u in (si.on_update if si and si.on_update else []):
                    if str(u.update_mode) not in ('sem-inc', 'sem-add-imm'):
                        nonmono.add(u.id)
        for stream in streams.values():
            floor = {}
            for i, X in enumerate(stream):
                si = X.sync_info
                if si is None or not si.on_wait:
                    continue
                mode_ok = lambda w: (str(w.wait_mode) == 'sem-ge-imm'
                                     and w.id not in nonmono)
                waits = []
                for w in si.on_wait:
                    if (mode_ok(w) and w.id in floor
                            and floor[w.id] >= w.wait_value):
                        continue  # already implied earlier in this stream
                    waits.append(w)
                moved = []
                if len(waits) > 1:
                    # only sem-ge waits are relocatable; sem-sub barrier
                    # ops must stay exactly where Tile put them
                    fixed = [w for w in waits if not mode_ok(w)]
                    movable = [w for w in waits if mode_ok(w)]
                    keep = fixed + movable[:max(0, 1 - len(fixed))]
                    maybe_move = movable[max(0, 1 - len(fixed)):]
                    for w in maybe_move:
                        placed = False
                        for k in range(i - 1, -1, -1):
                            C = stream[k]
                            if isinstance(C, stop_types):
                                break
                            csi = C.sync_info
                            if csi and csi.on_update:
                                break  # never cross a semaphore producer
                            cw = list(csi.on_wait) if csi and csi.on_wait else []
                            if cw or isinstance(
                                    C, mybir.InstUnconditionalBranch):
                                continue  # occupied/branch; keep scanning
                                          # (same-sequencer waits commute)
                            C.sync_info = mybir.SyncInfo(on_wait=[w],
                                                         on_update=[])
                            placed = True
                            break
                        if placed:
                            moved.append(w)
                        else:
                            keep.append(w)
                    waits = keep
                for w in list(waits) + moved:
                    if mode_ok(w):
                        floor[w.id] = max(floor.get(w.id, 0), w.wait_value)
                X.sync_info = mybir.SyncInfo(
                    on_wait=waits,
                    on_update=list(si.on_update) if si.on_update else [])
                if len(waits) > 1:
                    leftover.append((X.name, str(X.engine),
                                     type(X).__name__, len(waits)))
    # The PE gate ENGINE_NOPs carry AP operands purely for Tile dep
    # tracking; walrus's engine check rejects a nop with operands, so
    # strip them now (tile.py does the same for InstNoOp instructions).
    for blk in nc.m.functions[0].blocks:
        for inst in blk.instructions:
            if (isinstance(inst, mybir.InstISA) and (inst.ins or inst.outs)
                    and inst.op_name == 'ENGINE_NOP'):
                inst.ins = []
                inst.outs = []

    if report is not None:
        report.extend(leftover)
    elif leftover:
        raise RuntimeError(f"wait legalization failed for: {leftover}")


def build_nc(mm_dt=MM_DT):
    nc = bass.Bass()

    xT = nc.dram_tensor("xT", [H, T_CORE], F32, kind="ExternalInput")
    wT = nc.dram_tensor("wT", [H, E], F32, kind="ExternalInput")
    biasb = nc.dram_tensor("biasb", [P, E], F32, kind="ExternalInput")
    idx_out = nc.dram_tensor("idx_out", [T_CORE, TOP_K], U32, kind="ExternalOutput")
    w_out = nc.dram_tensor("w_out", [T_CORE, TOP_K], F32, kind="ExternalOutput")

    xT3 = xT.ap().rearrange("(c p) t -> p c t", p=P)      # [128, 32, 2048]
    wT3 = wT.ap().rearrange("(c p) e -> p c e", p=P)      # [128, 32, 256]
    idx3 = idx_out.ap().rearrange("(j p) k -> p j k", p=P)  # [128, 16, 8]
    w3 = w_out.ap().rearrange("(j p) k -> p j k", p=P)

    with tile.TileContext(nc) as tc:
        with (
            tc.tile_pool(name="const", bufs=1) as cpool,
            # bufs=8 so an xt slot's previous DMA sits 8 queue-round-robin
            # steps back -> same HWDGE queue -> WAW covered by queue FIFO,
            # leaving each xt DMA a single (PE slot-release) wait.
            tc.tile_pool(name="xin", bufs=8) as xpool,
            tc.tile_pool(name="psum", bufs=2, space="PSUM") as pspool,
            tc.tile_pool(name="work", bufs=2) as wpool,
        ):
            wsb = cpool.tile([P, KC, E], mm_dt)
            nc.sync.dma_start(wsb[:], wT3)
            bsb = cpool.tile([P, E], F32)
            nc.sync.dma_start(bsb[:], biasb.ap())
            # Persistent per-core output accumulators: no slot reuse, so
            # the DVE producers of idx/w never wait on output DMAs.
            idx_all = cpool.tile([P, TOK_TILES, TOP_K], U32)
            w_all = cpool.tile([P, TOK_TILES, TOP_K], F32)

            prev_sig = None
            prev_mm = None
            prev_dma = None
            last_wout = None
            for j in range(TOK_TILES):
                # ---- gate GEMM: logits[128 tok, 256 exp] ----
                xt = xpool.tile([P, KC, P], mm_dt, tag="xt")
                xt_dma = nc.sync.dma_start(xt[:], xT3[:, :, j * P:(j + 1) * P])
                _reserve(nc, nc.sync, xt_dma, 3, prev=prev_dma)
                prev_dma = xt_dma
                ps = pspool.tile([P, E], F32, tag="ps")
                # The fused fp32 matmul (self-loading LDWEIGHTS) only has
                # budget for ONE semaphore wait in walrus codegen, but the
                # tile-leading matmul needs the xt-DMA sem plus the
                # psum-slot-release sem. Emit a PE NoOp that declares those
                # data deps (1-elem APs, registered via annotate_deps) so
                # Tile's per-engine clock absorbs all waits there; the
                # matmuls then follow wait-free in PE program order. Tile
                # strips APs from InstNoOp at lowering, so walrus only
                # sees a plain NOP.
                gate = nc.tensor.nop(nofuse=True)
                gate.ins.ins = [nc.tensor.lower_ap(xt[0:1, 0, 0:1])]
                gate.ins.outs = [nc.tensor.lower_ap(ps[0:1, 0:1])]
                annotate_deps(tc.dep_state, gate.ins, tc.shadow_memory,
                              tc._rust_ctx, nc.inst_map)
                _reserve(nc, nc.tensor, gate, 4, prev=prev_mm)
                for c in range(KC):
                    mm = nc.tensor.matmul(
                        ps[:],
                        lhsT=xt[:, c, :],
                        rhs=wsb[:, c, :],
                        start=(c == 0),
                        stop=(c == KC - 1),
                    )
                prev_mm = mm

                # ---- scores / biased scores ----
                scores = wpool.tile([P, E], F32, tag="scores")
                sig = nc.scalar.activation(scores[:], ps[:], AF.Sigmoid)
                _reserve(nc, nc.scalar, sig, 3, prev=prev_sig)
                prev_sig = sig
                sfc = wpool.tile([P, E], F32, tag="sfc")
                badd = nc.vector.tensor_add(sfc[:], scores[:], bsb[:])
                _reserve(nc, nc.vector, badd, 3, prev=None)
                sfc3 = sfc[:].rearrange("p (g e) -> p g e", g=N_GROUP)

                # ---- group scores: sum of top-2 per group of 32 ----
                g3 = wpool.tile([P, N_GROUP, 8], F32, tag="g3")
                for g in range(N_GROUP):
                    nc.vector.max(g3[:, g, :], sfc[:, g * EG:(g + 1) * EG])
                gsum = wpool.tile([P, N_GROUP], F32, tag="gsum")
                nc.vector.tensor_add(gsum[:], g3[:, :, 0], g3[:, :, 1])

                # ---- pick top-4 groups; additive mask 0 / -BIG ----
                g8 = wpool.tile([P, 8], F32, tag="g8")
                nc.vector.max(g8[:], gsum[:])
                gneg = wpool.tile([P, N_GROUP], F32, tag="gneg")
                # (gsum < 4th-largest) * NEG -> 0 for kept groups, NEG else
                nc.vector.tensor_scalar(
                    gneg[:], gsum[:], g8[:, TOPK_GROUP - 1:TOPK_GROUP], NEG,
                    op0=OP.is_lt, op1=OP.mult,
                )

                # ---- masked biased scores; top-8 experts ----
                tmp = wpool.tile([P, E], F32, tag="tmp")
                tmp3 = tmp[:].rearrange("p (g e) -> p g e", g=N_GROUP)
                nc.vector.tensor_tensor(
                    tmp3, sfc3, gneg[:, :, None].to_broadcast([P, N_GROUP, EG]),
                    op=OP.add,
                )
                max8 = wpool.tile([P, 8], F32, tag="max8")
                nc.vector.max(max8[:], tmp[:])
                idx8 = idx_all[:, j, :]
                nc.vector.max_index(idx8, max8[:], tmp[:])

                # ---- selected-set mask via match_replace diff ----
                zap = wpool.tile([P, E], F32, tag="zap")
                nc.vector.match_replace(
                    zap[:], in_to_replace=max8[:], in_values=tmp[:], imm_value=NEG
                )
                sel = wpool.tile([P, E], U32, tag="sel")
                nc.vector.tensor_tensor(sel[:], tmp[:], zap[:], op=OP.not_equal)

                # ---- unbiased scores of the selected 8, sorted by score ----
                sm = wpool.tile([P, E], F32, tag="sm")
                nc.vector.memset(sm[:], NEG)
                nc.vector.copy_predicated(sm[:], sel[:], scores[:])
                smax8 = wpool.tile([P, 8], F32, tag="smax8")
                nc.vector.max(smax8[:], sm[:])
                sidx8 = wpool.tile([P, 8], U32, tag="sidx8")
                nc.vector.max_index(sidx8[:], smax8[:], sm[:])

                # ---- reorder scores to biased-rank order: w8[k] = sum_j
                #      smax8[j] * (sidx8[j] == idx8[k]) ----
                idxf = wpool.tile([P, 8], F32, tag="idxf")
                nc.vector.tensor_copy(idxf[:], idx8)
                sidxf = wpool.tile([P, 8], F32, tag="sidxf")
                nc.vector.tensor_copy(sidxf[:], sidx8[:])
                eq = wpool.tile([P, 8, 8], F32, tag="eq")
                nc.vector.tensor_tensor(
                    eq[:],
                    idxf[:, :, None].to_broadcast([P, 8, 8]),
                    sidxf[:, None, :].to_broadcast([P, 8, 8]),
                    op=OP.is_equal,
                )
                wprod = wpool.tile([P, 8, 8], F32, tag="wprod")
                nc.vector.tensor_tensor(
                    wprod[:], eq[:], smax8[:, None, :].to_broadcast([P, 8, 8]),
                    op=OP.mult,
                )
                w8 = wpool.tile([P, 8], F32, tag="w8")
                nc.vector.reduce_sum(w8[:], wprod[:], axis=mybir.AxisListType.X)

                # ---- normalize: w = 2.5 * w / (sum(w) + 1e-20) ----
                den = wpool.tile([P, 1], F32, tag="den")
                nc.vector.reduce_sum(den[:], w8[:], axis=mybir.AxisListType.X)
                nc.vector.tensor_scalar_add(den[:], den[:], 1e-20)
                rden = wpool.tile([P, 1], F32, tag="rden")
                nc.vector.reciprocal(rden[:], den[:])
                last_wout = nc.vector.tensor_scalar(
                    w_all[:, j, :], w8[:], rden[:], ROUTED_SCALING,
                    op0=OP.mult, op1=OP.mult,
                )

            d1 = nc.sync.dma_start(idx3, idx_all[:])
            _reserve(nc, nc.sync, d1, 2, prev=prev_dma)
            d2 = nc.sync.dma_start(w3, w_all[:])
            _reserve(nc, nc.sync, d2, 2, prev=d1)
            # Tail carriers: Tile's kernel-tail drain on SP waits on every
            # DMA queue sem (12 waits); give the legalizer enough nops.
            tail = d2.ins
            for _ in range(14):
                nop = nc.sync.nop(nofuse=True)
                add_dep_helper(nop.ins, tail, sync=False,
                               reason="tail drain wait carriers")
                tail = nop.ins

    _legalize_waits(nc)
    return nc


class _Runner:
    """Compile-once SPMD runner (mirrors bass2jax.run_bass_via_pjrt's
    multi-core path, but holds the jitted fn so repeated calls don't
    re-trace/re-jit; inputs can stay resident on device for timing)."""

    def __init__(self, nc):
        import jax
        from jax.experimental.shard_map import shard_map
        from jax.sharding import Mesh, NamedSharding, PartitionSpec

        from concourse import bass2jax

        bass2jax.install_neuronx_cc_hook()
        self._jax = jax
        self.nc = nc

        partition_name = (
            nc.partition_id_tensor.name if nc.partition_id_tensor else None
        )
        in_names, out_names, out_avals, zero_outs = [], [], [], []
        for alloc in nc.m.functions[0].allocations:
            if not isinstance(alloc, mybir.MemoryLocationSet):
                continue
            name = alloc.memorylocations[0].name
            if alloc.kind == "ExternalInput":
                if name != partition_name:
                    in_names.append(name)
            elif alloc.kind == "ExternalOutput":
                shape = tuple(alloc.tensor_shape)
                dtype = mybir.dt.np(alloc.dtype)
                out_names.append(name)
                out_avals.append(jax.core.ShapedArray(shape, dtype))
                zero_outs.append(np.zeros(shape, dtype))
        self.in_names = list(in_names)
        self.out_names = out_names
        self.out_avals = out_avals
        self.zero_outs = zero_outs
        n_params = len(in_names)
        self.n_params = n_params

        all_names = in_names + out_names
        if partition_name is not None:
            all_names.append(partition_name)

        def _body(*args):
            operands = list(args)
            if partition_name is not None:
                operands.append(bass2jax.partition_id_tensor())
            outs = bass2jax._bass_exec_p.bind(
                *operands,
                out_avals=tuple(out_avals),
                in_names=tuple(all_names),
                out_names=tuple(out_names),
                lowering_input_output_aliases=(),
                sim_require_finite=True,
                sim_require_nnan=True,
                nc=nc,
            )
            return tuple(outs)

        devices = jax.devices()[:N_CORES]
        assert len(devices) == N_CORES
        self.mesh = Mesh(np.asarray(devices), ("core",))
        n_outs = len(out_names)
        in_specs = (PartitionSpec("core"),) * (n_params + n_outs)
        out_specs = (PartitionSpec("core"),) * n_outs
        donate = tuple(range(n_params, n_params + n_outs))
        self._fn = jax.jit(
            shard_map(
                _body, mesh=self.mesh, in_specs=in_specs, out_specs=out_specs,
                check_rep=False,
            ),
            donate_argnums=donate,
            keep_unused=True,
        )
        self._sharding = NamedSharding(self.mesh, PartitionSpec("core"))

    def put_inputs(self, in_maps):
        """Concat per-core inputs on axis 0 and move to device once."""
        concat = [
            np.concatenate([np.asarray(m[name]) for m in in_maps], axis=0)
            for name in self.in_names
        ]
        return [self._jax.device_put(a, self._sharding) for a in concat]

    def _zeros(self):
        return [
            np.zeros((N_CORES * z.shape[0], *z.shape[1:]), z.dtype)
            for z in self.zero_outs
        ]

    def execute(self, dev_inputs):
        outs = self._fn(*dev_inputs, *self._zeros())
        self._jax.block_until_ready(outs)
        return outs

    def run(self, in_maps):
        dev_inputs = self.put_inputs(in_maps)
        out_arrs = self.execute(dev_inputs)
        return [
            {
                name: np.asarray(out_arrs[i]).reshape(
                    N_CORES, *self.out_avals[i].shape
                )[c]
                for i, name in enumerate(self.out_names)
            }
            for c in range(N_CORES)
        ]


_RUNNER_CACHE = {}


def _get_runner(mm_dt=MM_DT):
    if mm_dt not in _RUNNER_CACHE:
        _RUNNER_CACHE[mm_dt] = _Runner(build_nc(mm_dt))
    return _RUNNER_CACHE[mm_dt]


def make_in_maps(hidden_states, weight, e_score_correction_bias):
    x = np.ascontiguousarray(np.asarray(hidden_states), dtype=np.float32)
    x = x.reshape(T_FULL, H)
    w = np.asarray(weight, dtype=np.float32)
    b = np.asarray(e_score_correction_bias, dtype=np.float32)

    wT = np.ascontiguousarray(w.T)                       # [4096, 256]
    biasb = np.ascontiguousarray(np.broadcast_to(b, (P, E)))

    in_maps = []
    for i in range(N_CORES):
        xs = x[i * T_CORE:(i + 1) * T_CORE]
        in_maps.append({
            "xT": np.ascontiguousarray(xs.T),            # [4096, 2048]
            "wT": wT,
            "biasb": biasb,
        })
    return in_maps


def kernel(hidden_states, weight, e_score_correction_bias):
    runner = _get_runner()
    results = runner.run(
        make_in_maps(hidden_states, weight, e_score_correction_bias)
    )
    topk_idx = np.concatenate(
        [r["idx_out"].astype(np.int32) for r in results], axis=0
    )
    topk_weight = np.concatenate([r["w_out"] for r in results], axis=0)
    return topk_idx, topk_weight



# revision 39
# speedup vs baseline: 138.0991x; 138.0991x over previous
"""MiMo V2 MoE gate (sigmoid routing, grouped top-k) on 8 Trainium2 cores.

Contract: kernel(**inputs) takes the FULL unsharded inputs
(hidden_states [4,4096,4096] f32, weight [256,4096] f32,
e_score_correction_bias [256] f32) and returns (topk_idx int32 [16384,8],
topk_weight f32 [16384,8]) matching the MiMo V2 MoE gate reference
(sigmoid scores, bias-corrected grouped top-4-of-8 groups by top-2 sums,
top-8 experts, sum-normalized weights scaled by 2.5).

Strategy (data-parallel over tokens):
  - 16384 tokens sharded 2048/core across 8 NeuronCores.
  - Host pre-transposes x to [4096, 2048] per core and splits it into
    fp16 hi + fp16 lo halves (x = hi + lo to ~2^-22 relative), stacked
    in one DRAM tensor. The gate weight is replicated; by default it is
    kept exact as float32r and streamed as the MOVING matmul operand
    (full precision; the known fp32r precision loss is in the stationary
    LDWEIGHTS path, which we avoid), giving exact-rank routing with only
    2 PE passes instead of the 4-pass fp32 matmul.
  - Per core: PSUM-accumulated GEMM terms per 128-token tile, sigmoid on
    ScalarE, grouped top-k on VectorE (DVE sort8 primitives), weights
    recovered via a masked re-sort + 8x8 index match done BATCHED across
    all 16 tiles at the end.
"""

import sys

if "/opt/trn_rl_repo" not in sys.path:
    sys.path.insert(0, "/opt/trn_rl_repo")

import numpy as np

import concourse.bass as bass
import concourse.mybir as mybir
import concourse.tile as tile
from concourse.tile_rust import add_dep_helper, annotate_deps

P = 128
H = 4096
E = 256
N_CORES = 8
T_FULL = 16384
T_CORE = T_FULL // N_CORES  # 2048
KC = H // P                 # 32 contraction chunks per GEMM term
GT = 512                    # tokens per DMA group (1KB fp16 desc lines)
N_DMA_GROUPS = T_CORE // GT  # 4
TILES_PER_GROUP = GT // P   # 4
TOK_TILES = T_CORE // P     # 16 token tiles per core
N_GROUP = 8
EG = E // N_GROUP           # 32 experts per group
TOPK_GROUP = 4
TOP_K = 8
ROUTED_SCALING = 2.5
NEG = -1e30

F32 = mybir.dt.float32
F32R = mybir.dt.float32r
F16 = mybir.dt.float16
F8E5 = mybir.dt.float8e5
U32 = mybir.dt.uint32
AF = mybir.ActivationFunctionType
OP = mybir.AluOpType

CROSS_SCALE = 512.0  # 2^9: exact power-of-2 pre-scale for the fp8 cross
                     # pass so it shares one PSUM with the scaled hi pass


def _split16(a):
    """a (f32) -> (hi, lo) fp16 with hi + lo ~= a to ~2^-22 relative."""
    hi = a.astype(np.float16)
    lo = (a - hi.astype(np.float32)).astype(np.float16)
    return hi, lo


# A scheme is (name, x_dt, w_dt, n_x, n_w, terms, make_x, make_w) where
# terms is a list of (x_term_idx, w_term_idx) GEMM passes accumulated in
# PSUM, and make_x/make_w map the f32 host array -> list of term arrays.
SCHEMES = {
    # exact fp32 4-pass matmul (slow, bit-accurate baseline)
    "f32": dict(
        x_dt=F32, w_dt=F32, n_x=1, n_w=1, terms=((0, 0),),
        make_x=lambda x: [x.astype(np.float32)],
        make_w=lambda w: [w.astype(np.float32)],
    ),
    # x split hi+lo fp16 (stationary), w exact fp32 bits streamed as
    # float32r moving operand: 2 full-speed passes
    "xsplit_wr": dict(
        x_dt=F16, w_dt=F32R, n_x=2, n_w=1, terms=((0, 0), (1, 0)),
        make_x=lambda x: list(_split16(x)),
        make_w=lambda w: [w.astype(np.float32)],
    ),
    # x split hi+lo fp16, w split hi+lo fp16: 3 full-speed passes
    "split3": dict(
        x_dt=F16, w_dt=F16, n_x=2, n_w=2, terms=((0, 0), (1, 0), (0, 1)),
        make_x=lambda x: list(_split16(x)),
        make_w=lambda w: list(_split16(w)),
    ),
    # single-pass fp16 probe (fails idx tolerance; diagnostics only)
    "f16": dict(
        x_dt=F16, w_dt=F16, n_x=1, n_w=1, terms=((0, 0),),
        make_x=lambda x: [x.astype(np.float16)],
        make_w=lambda w: [w.astype(np.float16)],
    ),
    # x fp16 single (stationary), w float32r moving probe
    "x16_wr": dict(
        x_dt=F16, w_dt=F32R, n_x=1, n_w=1, terms=((0, 0),),
        make_x=lambda x: [x.astype(np.float16)],
        make_w=lambda w: [w.astype(np.float32)],
    ),
    # hi pass fp16 (x pre-scaled by 2^9) + both cross terms K-stacked in
    # one fp8e5m2 DoubleRow pass (also scaled 2^9); the 2^-9 descale is
    # folded into the sigmoid's scale operand. 1.5 pass-equivalents.
    "f8cross": dict(
        x_dt=F16, w_dt=F16, n_x=1, n_w=1, terms=((0, 0),),
        f8cross=True, sig_scale=1.0 / CROSS_SCALE,
        make_x=lambda x: [
            (x.astype(np.float16) * np.float16(CROSS_SCALE))],
        make_w=lambda w: [w.astype(np.float16)],
    ),
    # f8cross with the xh8 half of the cross operand derived ON DEVICE
    # (ScalarE fp16 -> fp8 cast, pipelined one DMA group ahead), saving
    # 8 MiB/core of DMA.
    "f8crossd": dict(
        x_dt=F16, w_dt=F16, n_x=1, n_w=1, terms=((0, 0),),
        f8cross=True, dev_cast=True, sig_scale=1.0 / CROSS_SCALE,
        make_x=lambda x: [
            (x.astype(np.float16) * np.float16(CROSS_SCALE))],
        make_w=lambda w: [w.astype(np.float16)],
    ),
}

SCHEME = "split3"


def _reserve(nc, eng, X, n, prev=None):
    """Emit n plain nops on X's engine, ordered after `prev` (a
    BassInstruction or None) and before X. They act as spare 1-wait
    carriers for _legalize_waits (every TPB instruction has exactly one
    HW wait slot; Tile can assign several waits to one instruction,
    which walrus then rejects)."""
    last = prev.ins if prev is not None else None
    for _ in range(n):
        nop = eng.nop(nofuse=True)
        if last is not None:
            add_dep_helper(nop.ins, last, sync=False,
                           reason="chain reserve nop after predecessor")
        add_dep_helper(X.ins, nop.ins, sync=False,
                       reason="reserve nop precedes its instruction")
        last = nop.ins


def _legalize_waits(nc, report=None):
    """Every TPB instruction has ONE hardware wait slot; Tile can assign
    several on_wait entries to an instruction, which walrus rejects
    ("Too many sync wait commands"). Fix in two ways, per engine stream
    (scheduled order):
      1. value-floor dedup: drop waits already implied by an earlier wait
         on the same semaphore in this stream (monotonic sems).
      2. excess-wait hoisting: move extra waits onto the nearest earlier
         wait-free instruction, scanning only across instructions with no
         on_update (pure nops) -- crossing an updater could reorder a
         producer chain and deadlock; this rule keeps placements provably
         safe. _reserve() plants such nops next to risky instructions.
    Drains are skipped (they encode multi-sem waits natively)."""
    stop_types = (
        mybir.InstDrain,
        mybir.InstEventSemaphore,
        mybir.InstCall,
    )
    leftover = []
    if True:
        # The kernel CFG is linear (main block -> end block), so per-engine
        # program order is the block-order concatenation. Crossing an
        # unconditional branch just means waiting before the jump.
        streams = {}
        nonmono = set()  # sems that ever decrease (barrier sems): no
                         # floor-dedup and no relocation for their waits
        for blk in nc.m.functions[0].blocks:
            for inst in blk.instructions:
                streams.setdefault(str(inst.engine), []).append(inst)
                si = inst.sync_info
                for u in (si.on_update if si and si.on_update else []):
                    if str(u.update_mode) not in ('sem-inc', 'sem-add-imm'):
                        nonmono.add(u.id)
        for stream in streams.values():
            floor = {}
            for i, X in enumerate(stream):
                si = X.sync_info
                if si is None or not si.on_wait:
                    continue
                mode_ok = lambda w: (str(w.wait_mode) == 'sem-ge-imm'
                                     and w.id not in nonmono)
                waits = []
                for w in si.on_wait:
                    if (mode_ok(w) and w.id in floor
                            and floor[w.id] >= w.wait_value):
                        continue  # already implied earlier in this stream
                    waits.append(w)
                moved = []
                if len(waits) > 1:
                    # only sem-ge waits are relocatable; sem-sub barrier
                    # ops must stay exactly where Tile put them
                    fixed = [w for w in waits if not mode_ok(w)]
                    movable = [w for w in waits if mode_ok(w)]
                    keep = fixed + movable[:max(0, 1 - len(fixed))]
                    maybe_move = movable[max(0, 1 - len(fixed)):]
                    for w in maybe_move:
                        placed = False
                        for k in range(i - 1, -1, -1):
                            C = stream[k]
                            if isinstance(C, stop_types):
                                break
                            csi = C.sync_info
                            if csi and csi.on_update:
                                break  # never cross a semaphore producer
                            cw = list(csi.on_wait) if csi and csi.on_wait else []
                            if cw or isinstance(
                                    C, mybir.InstUnconditionalBranch):
                                continue  # occupied/branch; keep scanning
                                          # (same-sequencer waits commute)
                            C.sync_info = mybir.SyncInfo(on_wait=[w],
                                                         on_update=[])
                            placed = True
                            break
                        if placed:
                            moved.append(w)
                        else:
                            keep.append(w)
                    waits = keep
                for w in list(waits) + moved:
                    if mode_ok(w):
                        floor[w.id] = max(floor.get(w.id, 0), w.wait_value)
                X.sync_info = mybir.SyncInfo(
                    on_wait=waits,
                    on_update=list(si.on_update) if si.on_update else [])
                if len(waits) > 1:
                    leftover.append((X.name, str(X.engine),
                                     type(X).__name__, len(waits)))
    # The PE gate ENGINE_NOPs carry AP operands purely for Tile dep
    # tracking; walrus's engine check rejects a nop with operands, so
    # strip them now (tile.py does the same for InstNoOp instructions).
    for blk in nc.m.functions[0].blocks:
        for inst in blk.instructions:
            if (isinstance(inst, mybir.InstISA) and (inst.ins or inst.outs)
                    and inst.op_name == 'ENGINE_NOP'):
                inst.ins = []
                inst.outs = []

    if report is not None:
        report.extend(leftover)
    elif leftover:
        raise RuntimeError(f"wait legalization failed for: {leftover}")


def build_nc(scheme=SCHEME, reps=1, stage="full"):
    """stage: 'full' | 'nodve' (gemm+sigmoid) | 'gemm' | 'dma' —
    partial builds for HW bottleneck isolation via reps-slope timing."""
    cfg = SCHEMES[scheme]
    x_dt, w_dt = cfg["x_dt"], cfg["w_dt"]
    n_x, n_w, terms = cfg["n_x"], cfg["n_w"], cfg["terms"]
    f8cross = cfg.get("f8cross", False)
    dev_cast = cfg.get("dev_cast", False)
    sig_scale = cfg.get("sig_scale", 1.0)

    nc = bass.Bass()

    # x is staged group-major on the host: [g, p, c, t] so each
    # partition's slice of a DMA group is ONE contiguous run (1-2 DMA
    # descriptors per partition instead of n_x*KC strided 1KB lines).
    xstk = nc.dram_tensor("xstk", [N_DMA_GROUPS * P, n_x * KC * GT], x_dt,
                          kind="ExternalInput")
    wstk = nc.dram_tensor("wstk", [n_w * H, E], w_dt, kind="ExternalInput")
    biasb = nc.dram_tensor("biasb", [P, E], F32, kind="ExternalInput")
    if f8cross:
        n_x8 = 1 if dev_cast else 2
        x8stk = nc.dram_tensor("x8stk",
                               [N_DMA_GROUPS * P, n_x8 * KC * GT], F8E5,
                               kind="ExternalInput")
        w8stk = nc.dram_tensor("w8stk", [2 * H, E], F8E5,
                               kind="ExternalInput")
        x84 = x8stk.ap().rearrange("(g p) (c t) -> g p c t",
                                   g=N_DMA_GROUPS, c=n_x8 * KC)
        w83 = w8stk.ap().rearrange("(c p) e -> p c e", p=P)  # [128, 64, 256]
    idx_out = nc.dram_tensor("idx_out", [T_CORE, TOP_K], U32, kind="ExternalOutput")
    w_out = nc.dram_tensor("w_out", [T_CORE, TOP_K], F32, kind="ExternalOutput")

    x4 = xstk.ap().rearrange("(g p) (c t) -> g p c t",
                             g=N_DMA_GROUPS, c=n_x * KC)
    w3 = wstk.ap().rearrange("(c p) e -> p c e", p=P)   # [128, n_w*32, 256]
    idx3 = idx_out.ap().rearrange("(j p) k -> p j k", p=P)  # [128, 16, 8]
    wo3 = w_out.ap().rearrange("(j p) k -> p j k", p=P)

    with tile.TileContext(nc) as tc:
        with (
            tc.tile_pool(name="const", bufs=1) as cpool,
            tc.tile_pool(name="xin", bufs=2) as xpool,
            tc.tile_pool(name="psum", bufs=8, space="PSUM") as pspool,
            tc.tile_pool(name="work", bufs=2) as wpool,
        ):
            wsb = cpool.tile([P, n_w * KC, E], w_dt)
            nc.sync.dma_start(wsb[:], w3)
            if f8cross:
                w8sb = cpool.tile([P, 2 * KC, E], F8E5)
                nc.sync.dma_start(w8sb[:], w83)
            bsb = cpool.tile([P, E], F32)
            nc.sync.dma_start(bsb[:], biasb.ap())
            negc = cpool.tile([P, E], F32)
            mset = nc.vector.memset(negc[:], NEG)
            # DVE gate: absorb the biasb-DMA wait once, up front, so the
            # first badd doesn't need two hardware wait slots.
            dve_gate = nc.vector.nop(nofuse=True)
            dve_gate.ins.ins = [nc.vector.lower_ap(bsb[0:1, 0:1])]
            dve_gate.ins.outs = []
            annotate_deps(tc.dep_state, dve_gate.ins, tc.shadow_memory,
                          tc._rust_ctx, nc.inst_map)
            # Persistent per-core tiles: no slot reuse, so producers never
            # wait on cross-tile consumers or output DMAs.
            idx_all = cpool.tile([P, TOK_TILES, TOP_K], U32)
            max8_all = cpool.tile([P, TOK_TILES, TOP_K], F32)
            smax8_all = cpool.tile([P, TOK_TILES, TOP_K], F32)
            sidx8_all = cpool.tile([P, TOK_TILES, TOP_K], U32)
            w_all = cpool.tile([P, TOK_TILES, TOP_K], F32)
            if stage != "full":
                # partial builds skip the producers; keep outputs defined
                for t in (idx_all, max8_all, smax8_all, sidx8_all, w_all):
                    nc.vector.memset(t[:], 0)

            prev_sig = None
            prev_mm = None
            prev_dma = None
            prev_dve = mset
            n_groups_total = reps * N_DMA_GROUPS
            group_tiles = {}

            def emit_group_load(gi):
                nonlocal prev_dma, prev_sig
                g = gi % N_DMA_GROUPS
                xg = xpool.tile([P, n_x * KC, GT], x_dt, tag="xg")
                d = nc.sync.dma_start(xg[:], x4[g])
                _reserve(nc, nc.sync, d, 6, prev=prev_dma)
                prev_dma = d
                xg8 = None
                if f8cross:
                    xg8 = xpool.tile([P, 2 * KC, GT], F8E5, tag="xg8")
                    if dev_cast:
                        d8 = nc.sync.dma_start(xg8[:, 0:KC, :], x84[g])
                    else:
                        d8 = nc.sync.dma_start(xg8[:], x84[g])
                    _reserve(nc, nc.sync, d8, 6, prev=prev_dma)
                    prev_dma = d8
                    if dev_cast:
                        # xg holds xh*2^9 fp16; undo the scale during the
                        # fp8 cast so xh8 pairs with wl*2^9.
                        cast = nc.scalar.activation(
                            xg8[:, KC:2 * KC, :], xg[:], AF.Copy,
                            scale=1.0 / CROSS_SCALE)
                        _reserve(nc, nc.scalar, cast, 3, prev=prev_sig)
                        prev_sig = cast
                group_tiles[gi] = (xg, xg8)

            emit_group_load(0)
            for gi in range(n_groups_total):
                if gi + 1 < n_groups_total:
                    emit_group_load(gi + 1)
                xg, xg8 = group_tiles.pop(gi)
                g = gi % N_DMA_GROUPS
                if stage == "dma":
                    continue
                for jj in range(TILES_PER_GROUP):
                    j = g * TILES_PER_GROUP + jj
                    ps = pspool.tile([P, E], F32, tag="ps")
                    # The fused matmul (self-loading LDWEIGHTS) only has
                    # budget for ONE semaphore wait in walrus codegen, but
                    # the tile-leading matmul needs the xg-DMA sem plus the
                    # psum-slot-release sem. Emit a PE NoOp that declares
                    # those data deps (1-elem APs, registered via
                    # annotate_deps) so Tile's per-engine clock absorbs all
                    # waits there; the matmuls then follow wait-free in PE
                    # program order. Tile strips APs from InstNoOp at
                    # lowering, so walrus only sees a plain NOP.
                    gate = nc.tensor.nop(nofuse=True)
                    gate.ins.ins = [nc.tensor.lower_ap(xg[0:1, 0, 0:1])]
                    if f8cross:
                        gate.ins.ins.append(
                            nc.tensor.lower_ap(xg8[0:1, 0, 0:1]))
                    gate.ins.outs = [nc.tensor.lower_ap(ps[0:1, 0:1])]
                    annotate_deps(tc.dep_state, gate.ins, tc.shadow_memory,
                                  tc._rust_ctx, nc.inst_map)
                    _reserve(nc, nc.tensor, gate, 4, prev=prev_mm)
                    n_mm = len(terms) * KC + (KC if f8cross else 0)
                    i_mm = 0
                    for (xi, wi) in terms:
                        for c in range(KC):
                            mm = nc.tensor.matmul(
                                ps[:],
                                lhsT=xg[:, xi * KC + c,
                                        jj * P:(jj + 1) * P],
                                rhs=wsb[:, wi * KC + c, :],
                                start=(i_mm == 0),
                                stop=(i_mm == n_mm - 1),
                            )
                            i_mm += 1
                    if f8cross:
                        for c in range(KC):
                            mm = nc.tensor.matmul(
                                ps[:],
                                lhsT=xg8[:, 2 * c:2 * c + 2,
                                         jj * P:(jj + 1) * P],
                                rhs=w8sb[:, 2 * c:2 * c + 2, :],
                                start=False,
                                stop=(i_mm == n_mm - 1),
                                perf_mode=mybir.MatmulPerfMode.DoubleRow,
                            )
                            i_mm += 1
                    prev_mm = mm
                    if stage == "gemm":
                        continue

                    # ---- scores / biased scores ----
                    scores = wpool.tile([P, E], F32, tag="scores")
                    sig = nc.scalar.activation(scores[:], ps[:], AF.Sigmoid,
                                               scale=sig_scale)
                    _reserve(nc, nc.scalar, sig, 3, prev=prev_sig)
                    prev_sig = sig
                    if stage == "nodve":
                        continue
                    sfc = wpool.tile([P, E], F32, tag="sfc")
                    badd = nc.vector.tensor_add(sfc[:], scores[:], bsb[:])
                    _reserve(nc, nc.vector, badd, 3, prev=prev_dve)
                    sfc3 = sfc[:].rearrange("p (g e) -> p g e", g=N_GROUP)

                    # ---- group scores: sum of top-2 per group of 32 ----
                    g3 = wpool.tile([P, N_GROUP, 8], F32, tag="g3")
                    for gg in range(N_GROUP):
                        nc.vector.max(g3[:, gg, :],
                                      sfc[:, gg * EG:(gg + 1) * EG])
                    gsum = wpool.tile([P, N_GROUP], F32, tag="gsum")
                    nc.vector.tensor_add(gsum[:], g3[:, :, 0], g3[:, :, 1])

                    # ---- pick top-4 groups; additive mask 0 / -BIG ----
                    g8 = wpool.tile([P, 8], F32, tag="g8")
                    nc.vector.max(g8[:], gsum[:])
                    gneg = wpool.tile([P, N_GROUP], F32, tag="gneg")
                    nc.vector.tensor_scalar(
                        gneg[:], gsum[:],
                        g8[:, TOPK_GROUP - 1:TOPK_GROUP], NEG,
                        op0=OP.is_lt, op1=OP.mult,
                    )

                    # ---- masked biased scores; top-8 experts ----
                    tmp = wpool.tile([P, E], F32, tag="tmp")
                    tmp3 = tmp[:].rearrange("p (g e) -> p g e", g=N_GROUP)
                    nc.vector.tensor_tensor(
                        tmp3, sfc3,
                        gneg[:, :, None].to_broadcast([P, N_GROUP, EG]),
                        op=OP.add,
                    )
                    max8 = max8_all[:, j, :]
                    nc.vector.max(max8, tmp[:])
                    nc.vector.max_index(idx_all[:, j, :], max8, tmp[:])

                    # ---- unbiased scores of the selected 8 ----
                    zap = wpool.tile([P, E], F32, tag="zap")
                    nc.vector.match_replace(
                        zap[:], in_to_replace=max8, in_values=tmp[:],
                        imm_value=NEG,
                    )
                    sel = wpool.tile([P, E], U32, tag="sel")
                    nc.vector.tensor_tensor(sel[:], tmp[:], zap[:],
                                            op=OP.not_equal)
                    sm = wpool.tile([P, E], F32, tag="sm")
                    nc.vector.select(sm[:], sel[:], scores[:], negc[:])
                    nc.vector.max(smax8_all[:, j, :], sm[:])
                    prev_dve = nc.vector.max_index(sidx8_all[:, j, :],
                                                   smax8_all[:, j, :], sm[:])

            if stage == "full":
                # ---- batched epilogue over all 16 tiles ----
                # reorder scores to biased-rank order:
                #   w8[j,k] = sum_r smax8[j,r] * (sidx8[j,r] == idx8[j,k])
                idxf = cpool.tile([P, TOK_TILES, TOP_K], F32)
                nc.vector.tensor_copy(idxf[:], idx_all[:])
                sidxf = cpool.tile([P, TOK_TILES, TOP_K], F32)
                nc.vector.tensor_copy(sidxf[:], sidx8_all[:])
                eq = cpool.tile([P, TOK_TILES, TOP_K, TOP_K], F32)
                nc.vector.tensor_tensor(
                    eq[:],
                    idxf[:, :, :, None].to_broadcast(
                        [P, TOK_TILES, TOP_K, TOP_K]),
                    sidxf[:, :, None, :].to_broadcast(
                        [P, TOK_TILES, TOP_K, TOP_K]),
                    op=OP.is_equal,
                )
                wprod = cpool.tile([P, TOK_TILES, TOP_K, TOP_K], F32)
                nc.vector.tensor_tensor(
                    wprod[:], eq[:],
                    smax8_all[:, :, None, :].to_broadcast(
                        [P, TOK_TILES, TOP_K, TOP_K]),
                    op=OP.mult,
                )
                w8 = cpool.tile([P, TOK_TILES, TOP_K], F32)
                nc.vector.reduce_sum(w8[:], wprod[:],
                                     axis=mybir.AxisListType.X)

                # ---- normalize: w = 2.5 * w / (sum(w) + 1e-20) ----
                den = cpool.tile([P, TOK_TILES], F32)
                nc.vector.reduce_sum(den[:], w8[:],
                                     axis=mybir.AxisListType.X)
                nc.vector.tensor_scalar_add(den[:], den[:], 1e-20)
                rden = cpool.tile([P, TOK_TILES], F32)
                nc.vector.reciprocal(rden[:], den[:])
                nc.vector.scalar_tensor_tensor(
                    w_all[:], w8[:], ROUTED_SCALING,
                    rden[:, :, None].to_broadcast([P, TOK_TILES, TOP_K]),
                    op0=OP.mult, op1=OP.mult,
                )

            d1 = nc.sync.dma_start(idx3, idx_all[:])
            _reserve(nc, nc.sync, d1, 2, prev=prev_dma)
            d2 = nc.sync.dma_start(wo3, w_all[:])
            _reserve(nc, nc.sync, d2, 2, prev=d1)
            # Tail carriers: Tile's kernel-tail drain on SP waits on every
            # DMA queue sem (12 waits); give the legalizer enough nops.
            tail = d2.ins
            for _ in range(14):
                nop = nc.sync.nop(nofuse=True)
                add_dep_helper(nop.ins, tail, sync=False,
                               reason="tail drain wait carriers")
                tail = nop.ins

    _legalize_waits(nc)
    return nc


class _Runner:
    """Compile-once SPMD runner (mirrors bass2jax.run_bass_via_pjrt's
    multi-core path, but holds the jitted fn so repeated calls don't
    re-trace/re-jit; inputs can stay resident on device for timing).
    With chain=K, one dispatch runs the NEFF K times back-to-back on
    device (chained through the output buffers), which lets test.py
    amortize away per-dispatch host/tunnel overhead and measure the
    per-execution hardware time as a slope."""

    def __init__(self, nc, chain=1, donate=True):
        import jax
        from jax.experimental.shard_map import shard_map
        from jax.sharding import Mesh, NamedSharding, PartitionSpec

        from concourse import bass2jax

        bass2jax.install_neuronx_cc_hook()
        self._jax = jax
        self.nc = nc
        self.chain = chain

        partition_name = (
            nc.partition_id_tensor.name if nc.partition_id_tensor else None
        )
        in_names, out_names, out_avals, zero_outs = [], [], [], []
        for alloc in nc.m.functions[0].allocations:
            if not isinstance(alloc, mybir.MemoryLocationSet):
                continue
            name = alloc.memorylocations[0].name
            if alloc.kind == "ExternalInput":
                if name != partition_name:
                    in_names.append(name)
            elif alloc.kind == "ExternalOutput":
                shape = tuple(alloc.tensor_shape)
                dtype = mybir.dt.np(alloc.dtype)
                out_names.append(name)
                out_avals.append(jax.core.ShapedArray(shape, dtype))
                zero_outs.append(np.zeros(shape, dtype))
        self.in_names = list(in_names)
        self.out_names = out_names
        self.out_avals = out_avals
        self.zero_outs = zero_outs
        n_params = len(in_names)
        self.n_params = n_params

        all_names = in_names + out_names
        if partition_name is not None:
            all_names.append(partition_name)

        def _body(*args):
            ins = list(args[:n_params])
            outs = list(args[n_params:])
            for _ in range(chain):
                operands = ins + list(outs)
                if partition_name is not None:
                    operands.append(bass2jax.partition_id_tensor())
                outs = bass2jax._bass_exec_p.bind(
                    *operands,
                    out_avals=tuple(out_avals),
                    in_names=tuple(all_names),
                    out_names=tuple(out_names),
                    lowering_input_output_aliases=(),
                    sim_require_finite=True,
                    sim_require_nnan=True,
                    nc=nc,
                )
            return tuple(outs)

        devices = jax.devices()[:N_CORES]
        assert len(devices) == N_CORES
        self.mesh = Mesh(np.asarray(devices), ("core",))
        n_outs = len(out_names)
        in_specs = (PartitionSpec("core"),) * (n_params + n_outs)
        out_specs = (PartitionSpec("core"),) * n_outs
        donate_nums = (
            tuple(range(n_params, n_params + n_outs)) if donate else ()
        )
        self._fn = jax.jit(
            shard_map(
                _body, mesh=self.mesh, in_specs=in_specs, out_specs=out_specs,
                check_rep=False,
            ),
            donate_argnums=donate_nums,
            keep_unused=True,
        )
        self._sharding = NamedSharding(self.mesh, PartitionSpec("core"))

    def put_inputs(self, in_maps):
        """Concat per-core inputs on axis 0 and move to device once."""
        concat = [
            np.concatenate([np.asarray(m[name]) for m in in_maps], axis=0)
            for name in self.in_names
        ]
        return [self._jax.device_put(a, self._sharding) for a in concat]

    def _zeros(self):
        return [
            np.zeros((N_CORES * z.shape[0], *z.shape[1:]), z.dtype)
            for z in self.zero_outs
        ]

    _zeros_host = _zeros

    def execute(self, dev_inputs):
        outs = self._fn(*dev_inputs, *self._zeros())
        self._jax.block_until_ready(outs)
        return outs

    def run(self, in_maps):
        dev_inputs = self.put_inputs(in_maps)
        out_arrs = self.execute(dev_inputs)
        return [
            {
                name: np.asarray(out_arrs[i]).reshape(
                    N_CORES, *self.out_avals[i].shape
                )[c]
                for i, name in enumerate(self.out_names)
            }
            for c in range(N_CORES)
        ]


_RUNNER_CACHE = {}


def _get_runner(scheme=SCHEME, chain=1):
    key = (scheme, chain)
    if key not in _RUNNER_CACHE:
        _RUNNER_CACHE[key] = _Runner(build_nc(scheme), chain=chain)
    return _RUNNER_CACHE[key]


def _get_runner_nodonate(scheme=SCHEME):
    key = (scheme, "nodonate")
    if key not in _RUNNER_CACHE:
        _RUNNER_CACHE[key] = _Runner(build_nc(scheme), donate=False)
    return _RUNNER_CACHE[key]


def make_in_maps(hidden_states, weight, e_score_correction_bias,
                 scheme=SCHEME):
    cfg = SCHEMES[scheme]
    f8cross = cfg.get("f8cross", False)
    f8np = mybir.dt.np(F8E5)
    x = np.ascontiguousarray(np.asarray(hidden_states), dtype=np.float32)
    x = x.reshape(T_FULL, H)
    w = np.asarray(weight, dtype=np.float32)
    b = np.asarray(e_score_correction_bias, dtype=np.float32)

    w_terms = cfg["make_w"](w)  # each [256, 4096]
    wstk = np.concatenate([wt.T for wt in w_terms], axis=0)
    wstk = np.ascontiguousarray(wstk, dtype=mybir.dt.np(cfg["w_dt"]))
    biasb = np.ascontiguousarray(np.broadcast_to(b, (P, E)))
    if f8cross:
        whT = w.T.astype(np.float16).astype(np.float32)   # [4096, 256]
        wlT = w.T.astype(np.float32) - whT
        w8stk = np.ascontiguousarray(np.concatenate([
            whT.astype(f8np),
            (wlT * CROSS_SCALE).astype(f8np),
        ], axis=0))

    def group_major(stk):
        """[n*H, T] -> [G*P, n*KC*GT]: per (group, partition) contiguous."""
        n_ci = stk.shape[0] // P
        a = stk.reshape(n_ci, P, N_DMA_GROUPS, GT)
        return np.ascontiguousarray(
            a.transpose(2, 1, 0, 3).reshape(N_DMA_GROUPS * P, n_ci * GT))

    in_maps = []
    for i in range(N_CORES):
        xs = np.ascontiguousarray(x[i * T_CORE:(i + 1) * T_CORE].T)
        x_terms = cfg["make_x"](xs)  # each [4096, 2048]
        xstk = np.concatenate(
            [xt.astype(mybir.dt.np(cfg["x_dt"])) for xt in x_terms], axis=0)
        m = {
            "xstk": group_major(xstk),
            "wstk": wstk,
            "biasb": biasb,
        }
        if f8cross:
            xh = xs.astype(np.float16).astype(np.float32)
            xl = xs - xh
            if cfg.get("dev_cast", False):
                x8 = (xl * CROSS_SCALE).astype(f8np)
            else:
                x8 = np.concatenate([
                    (xl * CROSS_SCALE).astype(f8np),
                    xh.astype(f8np),
                ], axis=0)
            m["x8stk"] = group_major(x8)
            m["w8stk"] = w8stk
        in_maps.append(m)
    return in_maps


def kernel(hidden_states, weight, e_score_correction_bias):
    runner = _get_runner()
    results = runner.run(
        make_in_maps(hidden_states, weight, e_score_correction_bias)
    )
    topk_idx = np.concatenate(
        [r["idx_out"].astype(np.int32) for r in results], axis=0
    )
    topk_weight = np.concatenate([r["w_out"] for r in results], axis=0)
    return topk_idx, topk_weight


# revision 45
# speedup vs baseline: 192.1597x; 1.3915x over previous
"""MiMo V2 MoE gate (sigmoid routing, grouped top-k) on 8 Trainium2 cores.

Contract: kernel(**inputs) takes the FULL unsharded inputs
(hidden_states [4,4096,4096] f32, weight [256,4096] f32,
e_score_correction_bias [256] f32) and returns (topk_idx int32 [16384,8],
topk_weight f32 [16384,8]) matching the MiMo V2 MoE gate reference
(sigmoid scores, bias-corrected grouped top-4-of-8 groups by top-2 sums,
top-8 experts, sum-normalized weights scaled by 2.5).

Strategy (data-parallel over tokens):
  - 16384 tokens sharded 2048/core across 8 NeuronCores.
  - Gate GEMM in 1.5 pass-equivalents ("f8crossd"): x and w split into
    fp16 hi + lo; the hi*hi pass runs in fp16 with x pre-scaled by 2^9
    (exact power-of-2), and BOTH cross terms (xl*wh + xh*wl) run as ONE
    K-stacked fp8e5m2 DoubleRow pass at 0.5 cycles/row, pre-scaled by
    2^9 so all passes share a single PSUM accumulation; the 2^-9
    descale folds into the sigmoid's scale operand (zero extra ops).
    The xh8 fp8 operand is derived on-device (ScalarE cast, pipelined
    one DMA group ahead), saving 8 MiB/core of DMA. Routing-rank error
    vs the fp32 reference: 23/131072 flipped indices (rel-err 9.5e-3,
    inside the 2e-2 gate; HW matches the host fp8 simulation exactly).
    Dropping to a single fp16/fp32r pass mis-ranks too many near-ties
    (225/152 flips, rel-err > 2e-2) and walrus forbids mixing 32-bit
    and 16-bit matmul operands, so this is the cheapest passing GEMM.
  - x is staged group-major ([group, partition, chunk, token]) so each
    partition's DMA-group slice is one contiguous run (minimal
    descriptor count, linear HBM walk).
  - Per core: PSUM-accumulated GEMM per 128-token tile, sigmoid on
    ScalarE, grouped top-k on VectorE (DVE sort8 primitives), weights
    recovered via a masked re-sort + 8x8 index match done BATCHED across
    all 16 tiles at the end.
  - CoreSim steady-state marginal: 89 us/exec (split3 alternative:
    164 us); HW reps-line measurements bound the real exec to sim-level
    (dispatch wall is flat in body replication to +-250 us resolution).
"""

import sys

if "/opt/trn_rl_repo" not in sys.path:
    sys.path.insert(0, "/opt/trn_rl_repo")

import numpy as np

import concourse.bass as bass
import concourse.mybir as mybir
import concourse.tile as tile
from concourse.tile_rust import add_dep_helper, annotate_deps

P = 128
H = 4096
E = 256
N_CORES = 8
T_FULL = 16384
T_CORE = T_FULL // N_CORES  # 2048
KC = H // P                 # 32 contraction chunks per GEMM term
GT = 512                    # tokens per DMA group (1KB fp16 desc lines)
N_DMA_GROUPS = T_CORE // GT  # 4
TILES_PER_GROUP = GT // P   # 4
TOK_TILES = T_CORE // P     # 16 token tiles per core
N_GROUP = 8
EG = E // N_GROUP           # 32 experts per group
TOPK_GROUP = 4
TOP_K = 8
ROUTED_SCALING = 2.5
NEG = -1e30

F32 = mybir.dt.float32
F32R = mybir.dt.float32r
F16 = mybir.dt.float16
F8E5 = mybir.dt.float8e5
U32 = mybir.dt.uint32
AF = mybir.ActivationFunctionType
OP = mybir.AluOpType

CROSS_SCALE = 512.0  # 2^9: exact power-of-2 pre-scale for the fp8 cross
                     # pass so it shares one PSUM with the scaled hi pass


def _split16(a):
    """a (f32) -> (hi, lo) fp16 with hi + lo ~= a to ~2^-22 relative."""
    hi = a.astype(np.float16)
    lo = (a - hi.astype(np.float32)).astype(np.float16)
    return hi, lo


# A scheme is (name, x_dt, w_dt, n_x, n_w, terms, make_x, make_w) where
# terms is a list of (x_term_idx, w_term_idx) GEMM passes accumulated in
# PSUM, and make_x/make_w map the f32 host array -> list of term arrays.
SCHEMES = {
    # exact fp32 4-pass matmul (slow, bit-accurate baseline)
    "f32": dict(
        x_dt=F32, w_dt=F32, n_x=1, n_w=1, terms=((0, 0),),
        make_x=lambda x: [x.astype(np.float32)],
        make_w=lambda w: [w.astype(np.float32)],
    ),
    # x split hi+lo fp16 (stationary), w exact fp32 bits streamed as
    # float32r moving operand: 2 full-speed passes
    "xsplit_wr": dict(
        x_dt=F16, w_dt=F32R, n_x=2, n_w=1, terms=((0, 0), (1, 0)),
        make_x=lambda x: list(_split16(x)),
        make_w=lambda w: [w.astype(np.float32)],
    ),
    # x split hi+lo fp16, w split hi+lo fp16: 3 full-speed passes
    "split3": dict(
        x_dt=F16, w_dt=F16, n_x=2, n_w=2, terms=((0, 0), (1, 0), (0, 1)),
        make_x=lambda x: list(_split16(x)),
        make_w=lambda w: list(_split16(w)),
    ),
    # single-pass fp16 probe (fails idx tolerance; diagnostics only)
    "f16": dict(
        x_dt=F16, w_dt=F16, n_x=1, n_w=1, terms=((0, 0),),
        make_x=lambda x: [x.astype(np.float16)],
        make_w=lambda w: [w.astype(np.float16)],
    ),
    # x fp16 single (stationary), w float32r moving probe
    "x16_wr": dict(
        x_dt=F16, w_dt=F32R, n_x=1, n_w=1, terms=((0, 0),),
        make_x=lambda x: [x.astype(np.float16)],
        make_w=lambda w: [w.astype(np.float32)],
    ),
    # hi pass fp16 (x pre-scaled by 2^9) + both cross terms K-stacked in
    # one fp8e5m2 DoubleRow pass (also scaled 2^9); the 2^-9 descale is
    # folded into the sigmoid's scale operand. 1.5 pass-equivalents.
    "f8cross": dict(
        x_dt=F16, w_dt=F16, n_x=1, n_w=1, terms=((0, 0),),
        f8cross=True, sig_scale=1.0 / CROSS_SCALE,
        make_x=lambda x: [
            (x.astype(np.float16) * np.float16(CROSS_SCALE))],
        make_w=lambda w: [w.astype(np.float16)],
    ),
    # f8cross with the xh8 half of the cross operand derived ON DEVICE
    # (ScalarE fp16 -> fp8 cast, pipelined one DMA group ahead), saving
    # 8 MiB/core of DMA.
    "f8crossd": dict(
        x_dt=F16, w_dt=F16, n_x=1, n_w=1, terms=((0, 0),),
        f8cross=True, dev_cast=True, sig_scale=1.0 / CROSS_SCALE,
        make_x=lambda x: [
            (x.astype(np.float16) * np.float16(CROSS_SCALE))],
        make_w=lambda w: [w.astype(np.float16)],
    ),
}

SCHEME = "f8crossd"


def _reserve(nc, eng, X, n, prev=None):
    """Emit n plain nops on X's engine, ordered after `prev` (a
    BassInstruction or None) and before X. They act as spare 1-wait
    carriers for _legalize_waits (every TPB instruction has exactly one
    HW wait slot; Tile can assign several waits to one instruction,
    which walrus then rejects)."""
    last = prev.ins if prev is not None else None
    for _ in range(n):
        nop = eng.nop(nofuse=True)
        if last is not None:
            add_dep_helper(nop.ins, last, sync=False,
                           reason="chain reserve nop after predecessor")
        add_dep_helper(X.ins, nop.ins, sync=False,
                       reason="reserve nop precedes its instruction")
        last = nop.ins


def _legalize_waits(nc, report=None):
    """Every TPB instruction has ONE hardware wait slot; Tile can assign
    several on_wait entries to an instruction, which walrus rejects
    ("Too many sync wait commands"). Fix in two ways, per engine stream
    (scheduled order):
      1. value-floor dedup: drop waits already implied by an earlier wait
         on the same semaphore in this stream (monotonic sems).
      2. excess-wait hoisting: move extra waits onto the nearest earlier
         wait-free instruction, scanning only across instructions with no
         on_update (pure nops) -- crossing an updater could reorder a
         producer chain and deadlock; this rule keeps placements provably
         safe. _reserve() plants such nops next to risky instructions.
    Drains are skipped (they encode multi-sem waits natively)."""
    stop_types = (
        mybir.InstDrain,
        mybir.InstEventSemaphore,
        mybir.InstCall,
    )
    leftover = []
    if True:
        # The kernel CFG is linear (main block -> end block), so per-engine
        # program order is the block-order concatenation. Crossing an
        # unconditional branch just means waiting before the jump.
        streams = {}
        nonmono = set()  # sems that ever decrease (barrier sems): no
                         # floor-dedup and no relocation for their waits
        for blk in nc.m.functions[0].blocks:
            for inst in blk.instructions:
                streams.setdefault(str(inst.engine), []).append(inst)
                si = inst.sync_info
                for u in (si.on_update if si and si.on_update else []):
                    if str(u.update_mode) not in ('sem-inc', 'sem-add-imm'):
                        nonmono.add(u.id)
        for stream in streams.values():
            floor = {}
            for i, X in enumerate(stream):
                si = X.sync_info
                if si is None or not si.on_wait:
                    continue
                mode_ok = lambda w: (str(w.wait_mode) == 'sem-ge-imm'
                                     and w.id not in nonmono)
                waits = []
                for w in si.on_wait:
                    if (mode_ok(w) and w.id in floor
                            and floor[w.id] >= w.wait_value):
                        continue  # already implied earlier in this stream
                    waits.append(w)
                moved = []
                if len(waits) > 1:
                    # only sem-ge waits are relocatable; sem-sub barrier
                    # ops must stay exactly where Tile put them
                    fixed = [w for w in waits if not mode_ok(w)]
                    movable = [w for w in waits if mode_ok(w)]
                    keep = fixed + movable[:max(0, 1 - len(fixed))]
                    maybe_move = movable[max(0, 1 - len(fixed)):]
                    for w in maybe_move:
                        placed = False
                        for k in range(i - 1, -1, -1):
                            C = stream[k]
                            if isinstance(C, stop_types):
                                break
                            csi = C.sync_info
                            if csi and csi.on_update:
                                break  # never cross a semaphore producer
                            cw = list(csi.on_wait) if csi and csi.on_wait else []
                            if cw or isinstance(
                                    C, mybir.InstUnconditionalBranch):
                                continue  # occupied/branch; keep scanning
                                          # (same-sequencer waits commute)
                            C.sync_info = mybir.SyncInfo(on_wait=[w],
                                                         on_update=[])
                            placed = True
                            break
                        if placed:
                            moved.append(w)
                        else:
                            keep.append(w)
                    waits = keep
                for w in list(waits) + moved:
                    if mode_ok(w):
                        floor[w.id] = max(floor.get(w.id, 0), w.wait_value)
                X.sync_info = mybir.SyncInfo(
                    on_wait=waits,
                    on_update=list(si.on_update) if si.on_update else [])
                if len(waits) > 1:
                    leftover.append((X.name, str(X.engine),
                                     type(X).__name__, len(waits)))
    # The PE gate ENGINE_NOPs carry AP operands purely for Tile dep
    # tracking; walrus's engine check rejects a nop with operands, so
    # strip them now (tile.py does the same for InstNoOp instructions).
    for blk in nc.m.functions[0].blocks:
        for inst in blk.instructions:
            if (isinstance(inst, mybir.InstISA) and (inst.ins or inst.outs)
                    and inst.op_name == 'ENGINE_NOP'):
                inst.ins = []
                inst.outs = []

    if report is not None:
        report.extend(leftover)
    elif leftover:
        raise RuntimeError(f"wait legalization failed for: {leftover}")


def build_nc(scheme=SCHEME, reps=1, stage="full"):
    """stage: 'full' | 'nodve' (gemm+sigmoid) | 'gemm' | 'dma' —
    partial builds for HW bottleneck isolation via reps-slope timing."""
    cfg = SCHEMES[scheme]
    x_dt, w_dt = cfg["x_dt"], cfg["w_dt"]
    n_x, n_w, terms = cfg["n_x"], cfg["n_w"], cfg["terms"]
    f8cross = cfg.get("f8cross", False)
    dev_cast = cfg.get("dev_cast", False)
    sig_scale = cfg.get("sig_scale", 1.0)

    nc = bass.Bass()

    # x is staged group-major on the host: [g, p, c, t] so each
    # partition's slice of a DMA group is ONE contiguous run (1-2 DMA
    # descriptors per partition instead of n_x*KC strided 1KB lines).
    xstk = nc.dram_tensor("xstk", [N_DMA_GROUPS * P, n_x * KC * GT], x_dt,
                          kind="ExternalInput")
    wstk = nc.dram_tensor("wstk", [n_w * H, E], w_dt, kind="ExternalInput")
    biasb = nc.dram_tensor("biasb", [P, E], F32, kind="ExternalInput")
    if f8cross:
        n_x8 = 1 if dev_cast else 2
        x8stk = nc.dram_tensor("x8stk",
                               [N_DMA_GROUPS * P, n_x8 * KC * GT], F8E5,
                               kind="ExternalInput")
        w8stk = nc.dram_tensor("w8stk", [2 * H, E], F8E5,
                               kind="ExternalInput")
        x84 = x8stk.ap().rearrange("(g p) (c t) -> g p c t",
                                   g=N_DMA_GROUPS, c=n_x8 * KC)
        w83 = w8stk.ap().rearrange("(c p) e -> p c e", p=P)  # [128, 64, 256]
    idx_out = nc.dram_tensor("idx_out", [T_CORE, TOP_K], U32, kind="ExternalOutput")
    w_out = nc.dram_tensor("w_out", [T_CORE, TOP_K], F32, kind="ExternalOutput")

    x4 = xstk.ap().rearrange("(g p) (c t) -> g p c t",
                             g=N_DMA_GROUPS, c=n_x * KC)
    w3 = wstk.ap().rearrange("(c p) e -> p c e", p=P)   # [128, n_w*32, 256]
    idx3 = idx_out.ap().rearrange("(j p) k -> p j k", p=P)  # [128, 16, 8]
    wo3 = w_out.ap().rearrange("(j p) k -> p j k", p=P)

    with tile.TileContext(nc) as tc:
        with (
            tc.tile_pool(name="const", bufs=1) as cpool,
            tc.tile_pool(name="xin", bufs=2) as xpool,
            tc.tile_pool(name="psum", bufs=8, space="PSUM") as pspool,
            tc.tile_pool(name="work", bufs=2) as wpool,
        ):
            wsb = cpool.tile([P, n_w * KC, E], w_dt)
            nc.sync.dma_start(wsb[:], w3)
            if f8cross:
                w8sb = cpool.tile([P, 2 * KC, E], F8E5)
                nc.sync.dma_start(w8sb[:], w83)
            bsb = cpool.tile([P, E], F32)
            nc.sync.dma_start(bsb[:], biasb.ap())
            negc = cpool.tile([P, E], F32)
            mset = nc.vector.memset(negc[:], NEG)
            # DVE gate: absorb the biasb-DMA wait once, up front, so the
            # first badd doesn't need two hardware wait slots.
            dve_gate = nc.vector.nop(nofuse=True)
            dve_gate.ins.ins = [nc.vector.lower_ap(bsb[0:1, 0:1])]
            dve_gate.ins.outs = []
            annotate_deps(tc.dep_state, dve_gate.ins, tc.shadow_memory,
                          tc._rust_ctx, nc.inst_map)
            # Persistent per-core tiles: no slot reuse, so producers never
            # wait on cross-tile consumers or output DMAs.
            idx_all = cpool.tile([P, TOK_TILES, TOP_K], U32)
            max8_all = cpool.tile([P, TOK_TILES, TOP_K], F32)
            smax8_all = cpool.tile([P, TOK_TILES, TOP_K], F32)
            sidx8_all = cpool.tile([P, TOK_TILES, TOP_K], U32)
            w_all = cpool.tile([P, TOK_TILES, TOP_K], F32)
            if stage != "full":
                # partial builds skip the producers; keep outputs defined
                for t in (idx_all, max8_all, smax8_all, sidx8_all, w_all):
                    nc.vector.memset(t[:], 0)

            prev_sig = None
            prev_mm = None
            prev_dma = None
            prev_dve = mset
            n_groups_total = reps * N_DMA_GROUPS
            group_tiles = {}

            def emit_group_load(gi):
                nonlocal prev_dma, prev_sig
                g = gi % N_DMA_GROUPS
                xg = xpool.tile([P, n_x * KC, GT], x_dt, tag="xg")
                d = nc.sync.dma_start(xg[:], x4[g])
                _reserve(nc, nc.sync, d, 6, prev=prev_dma)
                prev_dma = d
                xg8 = None
                if f8cross:
                    xg8 = xpool.tile([P, 2 * KC, GT], F8E5, tag="xg8")
                    if dev_cast:
                        d8 = nc.sync.dma_start(xg8[:, 0:KC, :], x84[g])
                    else:
                        d8 = nc.sync.dma_start(xg8[:], x84[g])
                    _reserve(nc, nc.sync, d8, 6, prev=prev_dma)
                    prev_dma = d8
                    if dev_cast:
                        # xg holds xh*2^9 fp16; undo the scale during the
                        # fp8 cast so xh8 pairs with wl*2^9.
                        cast = nc.scalar.activation(
                            xg8[:, KC:2 * KC, :], xg[:], AF.Copy,
                            scale=1.0 / CROSS_SCALE)
                        _reserve(nc, nc.scalar, cast, 3, prev=prev_sig)
                        prev_sig = cast
                group_tiles[gi] = (xg, xg8)

            emit_group_load(0)
            for gi in range(n_groups_total):
                if gi + 1 < n_groups_total:
                    emit_group_load(gi + 1)
                xg, xg8 = group_tiles.pop(gi)
                g = gi % N_DMA_GROUPS
                if stage == "dma":
                    continue
                for jj in range(TILES_PER_GROUP):
                    j = g * TILES_PER_GROUP + jj
                    ps = pspool.tile([P, E], F32, tag="ps")
                    # The fused matmul (self-loading LDWEIGHTS) only has
                    # budget for ONE semaphore wait in walrus codegen, but
                    # the tile-leading matmul needs the xg-DMA sem plus the
                    # psum-slot-release sem. Emit a PE NoOp that declares
                    # those data deps (1-elem APs, registered via
                    # annotate_deps) so Tile's per-engine clock absorbs all
                    # waits there; the matmuls then follow wait-free in PE
                    # program order. Tile strips APs from InstNoOp at
                    # lowering, so walrus only sees a plain NOP.
                    gate = nc.tensor.nop(nofuse=True)
                    gate.ins.ins = [nc.tensor.lower_ap(xg[0:1, 0, 0:1])]
                    if f8cross:
                        gate.ins.ins.append(
                            nc.tensor.lower_ap(xg8[0:1, 0, 0:1]))
                    gate.ins.outs = [nc.tensor.lower_ap(ps[0:1, 0:1])]
                    annotate_deps(tc.dep_state, gate.ins, tc.shadow_memory,
                                  tc._rust_ctx, nc.inst_map)
                    _reserve(nc, nc.tensor, gate, 4, prev=prev_mm)
                    n_mm = len(terms) * KC + (KC if f8cross else 0)
                    i_mm = 0
                    for (xi, wi) in terms:
                        for c in range(KC):
                            mm = nc.tensor.matmul(
                                ps[:],
                                lhsT=xg[:, xi * KC + c,
                                        jj * P:(jj + 1) * P],
                                rhs=wsb[:, wi * KC + c, :],
                                start=(i_mm == 0),
                                stop=(i_mm == n_mm - 1),
                            )
                            i_mm += 1
                    if f8cross:
                        for c in range(KC):
                            mm = nc.tensor.matmul(
                                ps[:],
                                lhsT=xg8[:, 2 * c:2 * c + 2,
                                         jj * P:(jj + 1) * P],
                                rhs=w8sb[:, 2 * c:2 * c + 2, :],
                                start=False,
                                stop=(i_mm == n_mm - 1),
                                perf_mode=mybir.MatmulPerfMode.DoubleRow,
                            )
                            i_mm += 1
                    prev_mm = mm
                    if stage == "gemm":
                        continue

                    # ---- scores / biased scores ----
                    scores = wpool.tile([P, E], F32, tag="scores")
                    sig = nc.scalar.activation(scores[:], ps[:], AF.Sigmoid,
                                               scale=sig_scale)
                    _reserve(nc, nc.scalar, sig, 3, prev=prev_sig)
                    prev_sig = sig
                    if stage == "nodve":
                        continue
                    sfc = wpool.tile([P, E], F32, tag="sfc")
                    badd = nc.vector.tensor_add(sfc[:], scores[:], bsb[:])
                    _reserve(nc, nc.vector, badd, 3, prev=prev_dve)
                    sfc3 = sfc[:].rearrange("p (g e) -> p g e", g=N_GROUP)

                    # ---- group scores: sum of top-2 per group of 32 ----
                    g3 = wpool.tile([P, N_GROUP, 8], F32, tag="g3")
                    for gg in range(N_GROUP):
                        nc.vector.max(g3[:, gg, :],
                                      sfc[:, gg * EG:(gg + 1) * EG])
                    gsum = wpool.tile([P, N_GROUP], F32, tag="gsum")
                    nc.vector.tensor_add(gsum[:], g3[:, :, 0], g3[:, :, 1])

                    # ---- pick top-4 groups; additive mask 0 / -BIG ----
                    g8 = wpool.tile([P, 8], F32, tag="g8")
                    nc.vector.max(g8[:], gsum[:])
                    gneg = wpool.tile([P, N_GROUP], F32, tag="gneg")
                    nc.vector.tensor_scalar(
                        gneg[:], gsum[:],
                        g8[:, TOPK_GROUP - 1:TOPK_GROUP], NEG,
                        op0=OP.is_lt, op1=OP.mult,
                    )

                    # ---- masked biased scores; top-8 experts ----
                    tmp = wpool.tile([P, E], F32, tag="tmp")
                    tmp3 = tmp[:].rearrange("p (g e) -> p g e", g=N_GROUP)
                    nc.vector.tensor_tensor(
                        tmp3, sfc3,
                        gneg[:, :, None].to_broadcast([P, N_GROUP, EG]),
                        op=OP.add,
                    )
                    max8 = max8_all[:, j, :]
                    nc.vector.max(max8, tmp[:])
                    nc.vector.max_index(idx_all[:, j, :], max8, tmp[:])

                    # ---- unbiased scores of the selected 8 ----
                    zap = wpool.tile([P, E], F32, tag="zap")
                    nc.vector.match_replace(
                        zap[:], in_to_replace=max8, in_values=tmp[:],
                        imm_value=NEG,
                    )
                    sel = wpool.tile([P, E], U32, tag="sel")
                    nc.vector.tensor_tensor(sel[:], tmp[:], zap[:],
                                            op=OP.not_equal)
                    sm = wpool.tile([P, E], F32, tag="sm")
                    nc.vector.select(sm[:], sel[:], scores[:], negc[:])
                    nc.vector.max(smax8_all[:, j, :], sm[:])
                    prev_dve = nc.vector.max_index(sidx8_all[:, j, :],
                                                   smax8_all[:, j, :], sm[:])

            if stage == "full":
                # ---- batched epilogue over all 16 tiles ----
                # reorder scores to biased-rank order:
                #   w8[j,k] = sum_r smax8[j,r] * (sidx8[j,r] == idx8[j,k])
                idxf = cpool.tile([P, TOK_TILES, TOP_K], F32)
                nc.vector.tensor_copy(idxf[:], idx_all[:])
                sidxf = cpool.tile([P, TOK_TILES, TOP_K], F32)
                nc.vector.tensor_copy(sidxf[:], sidx8_all[:])
                eq = cpool.tile([P, TOK_TILES, TOP_K, TOP_K], F32)
                nc.vector.tensor_tensor(
                    eq[:],
                    idxf[:, :, :, None].to_broadcast(
                        [P, TOK_TILES, TOP_K, TOP_K]),
                    sidxf[:, :, None, :].to_broadcast(
                        [P, TOK_TILES, TOP_K, TOP_K]),
                    op=OP.is_equal,
                )
                wprod = cpool.tile([P, TOK_TILES, TOP_K, TOP_K], F32)
                nc.vector.tensor_tensor(
                    wprod[:], eq[:],
                    smax8_all[:, :, None, :].to_broadcast(
                        [P, TOK_TILES, TOP_K, TOP_K]),
                    op=OP.mult,
                )
                w8 = cpool.tile([P, TOK_TILES, TOP_K], F32)
                nc.vector.reduce_sum(w8[:], wprod[:],
                                     axis=mybir.AxisListType.X)

                # ---- normalize: w = 2.5 * w / (sum(w) + 1e-20) ----
                den = cpool.tile([P, TOK_TILES], F32)
                nc.vector.reduce_sum(den[:], w8[:],
                                     axis=mybir.AxisListType.X)
                nc.vector.tensor_scalar_add(den[:], den[:], 1e-20)
                rden = cpool.tile([P, TOK_TILES], F32)
                nc.vector.reciprocal(rden[:], den[:])
                nc.vector.scalar_tensor_tensor(
                    w_all[:], w8[:], ROUTED_SCALING,
                    rden[:, :, None].to_broadcast([P, TOK_TILES, TOP_K]),
                    op0=OP.mult, op1=OP.mult,
                )

            d1 = nc.sync.dma_start(idx3, idx_all[:])
            _reserve(nc, nc.sync, d1, 2, prev=prev_dma)
            d2 = nc.sync.dma_start(wo3, w_all[:])
            _reserve(nc, nc.sync, d2, 2, prev=d1)
            # Tail carriers: Tile's kernel-tail drain on SP waits on every
            # DMA queue sem (12 waits); give the legalizer enough nops.
            tail = d2.ins
            for _ in range(14):
                nop = nc.sync.nop(nofuse=True)
                add_dep_helper(nop.ins, tail, sync=False,
                               reason="tail drain wait carriers")
                tail = nop.ins

    _legalize_waits(nc)
    return nc


class _Runner:
    """Compile-once SPMD runner (mirrors bass2jax.run_bass_via_pjrt's
    multi-core path, but holds the jitted fn so repeated calls don't
    re-trace/re-jit; inputs can stay resident on device for timing).
    With chain=K, one dispatch runs the NEFF K times back-to-back on
    device (chained through the output buffers), which lets test.py
    amortize away per-dispatch host/tunnel overhead and measure the
    per-execution hardware time as a slope."""

    def __init__(self, nc, chain=1, donate=True):
        import jax
        from jax.experimental.shard_map import shard_map
        from jax.sharding import Mesh, NamedSharding, PartitionSpec

        from concourse import bass2jax

        bass2jax.install_neuronx_cc_hook()
        self._jax = jax
        self.nc = nc
        self.chain = chain

        partition_name = (
            nc.partition_id_tensor.name if nc.partition_id_tensor else None
        )
        in_names, out_names, out_avals, zero_outs = [], [], [], []
        for alloc in nc.m.functions[0].allocations:
            if not isinstance(alloc, mybir.MemoryLocationSet):
                continue
            name = alloc.memorylocations[0].name
            if alloc.kind == "ExternalInput":
                if name != partition_name:
                    in_names.append(name)
            elif alloc.kind == "ExternalOutput":
                shape = tuple(alloc.tensor_shape)
                dtype = mybir.dt.np(alloc.dtype)
                out_names.append(name)
                out_avals.append(jax.core.ShapedArray(shape, dtype))
                zero_outs.append(np.zeros(shape, dtype))
        self.in_names = list(in_names)
        self.out_names = out_names
        self.out_avals = out_avals
        self.zero_outs = zero_outs
        n_params = len(in_names)
        self.n_params = n_params

        all_names = in_names + out_names
        if partition_name is not None:
            all_names.append(partition_name)

        def _body(*args):
            ins = list(args[:n_params])
            outs = list(args[n_params:])
            for _ in range(chain):
                operands = ins + list(outs)
                if partition_name is not None:
                    operands.append(bass2jax.partition_id_tensor())
                outs = bass2jax._bass_exec_p.bind(
                    *operands,
                    out_avals=tuple(out_avals),
                    in_names=tuple(all_names),
                    out_names=tuple(out_names),
                    lowering_input_output_aliases=(),
                    sim_require_finite=True,
                    sim_require_nnan=True,
                    nc=nc,
                )
            return tuple(outs)

        devices = jax.devices()[:N_CORES]
        assert len(devices) == N_CORES
        self.mesh = Mesh(np.asarray(devices), ("core",))
        n_outs = len(out_names)
        in_specs = (PartitionSpec("core"),) * (n_params + n_outs)
        out_specs = (PartitionSpec("core"),) * n_outs
        donate_nums = (
            tuple(range(n_params, n_params + n_outs)) if donate else ()
        )
        self._fn = jax.jit(
            shard_map(
                _body, mesh=self.mesh, in_specs=in_specs, out_specs=out_specs,
                check_rep=False,
            ),
            donate_argnums=donate_nums,
            keep_unused=True,
        )
        self._sharding = NamedSharding(self.mesh, PartitionSpec("core"))

    def put_inputs(self, in_maps):
        """Concat per-core inputs on axis 0 and move to device once."""
        concat = [
            np.concatenate([np.asarray(m[name]) for m in in_maps], axis=0)
            for name in self.in_names
        ]
        return [self._jax.device_put(a, self._sharding) for a in concat]

    def _zeros(self):
        return [
            np.zeros((N_CORES * z.shape[0], *z.shape[1:]), z.dtype)
            for z in self.zero_outs
        ]

    _zeros_host = _zeros

    def execute(self, dev_inputs):
        outs = self._fn(*dev_inputs, *self._zeros())
        self._jax.block_until_ready(outs)
        return outs

    def run(self, in_maps):
        dev_inputs = self.put_inputs(in_maps)
        out_arrs = self.execute(dev_inputs)
        return [
            {
                name: np.asarray(out_arrs[i]).reshape(
                    N_CORES, *self.out_avals[i].shape
                )[c]
                for i, name in enumerate(self.out_names)
            }
            for c in range(N_CORES)
        ]


_RUNNER_CACHE = {}


def _get_runner(scheme=SCHEME, chain=1):
    key = (scheme, chain)
    if key not in _RUNNER_CACHE:
        _RUNNER_CACHE[key] = _Runner(build_nc(scheme), chain=chain)
    return _RUNNER_CACHE[key]


def _get_runner_nodonate(scheme=SCHEME):
    key = (scheme, "nodonate")
    if key not in _RUNNER_CACHE:
        _RUNNER_CACHE[key] = _Runner(build_nc(scheme), donate=False)
    return _RUNNER_CACHE[key]


def make_in_maps(hidden_states, weight, e_score_correction_bias,
                 scheme=SCHEME):
    cfg = SCHEMES[scheme]
    f8cross = cfg.get("f8cross", False)
    f8np = mybir.dt.np(F8E5)
    x = np.ascontiguousarray(np.asarray(hidden_states), dtype=np.float32)
    x = x.reshape(T_FULL, H)
    w = np.asarray(weight, dtype=np.float32)
    b = np.asarray(e_score_correction_bias, dtype=np.float32)

    w_terms = cfg["make_w"](w)  # each [256, 4096]
    wstk = np.concatenate([wt.T for wt in w_terms], axis=0)
    wstk = np.ascontiguousarray(wstk, dtype=mybir.dt.np(cfg["w_dt"]))
    biasb = np.ascontiguousarray(np.broadcast_to(b, (P, E)))
    if f8cross:
        whT = w.T.astype(np.float16).astype(np.float32)   # [4096, 256]
        wlT = w.T.astype(np.float32) - whT
        w8stk = np.ascontiguousarray(np.concatenate([
            whT.astype(f8np),
            (wlT * CROSS_SCALE).astype(f8np),
        ], axis=0))

    def group_major(stk):
        """[n*H, T] -> [G*P, n*KC*GT]: per (group, partition) contiguous."""
        n_ci = stk.shape[0] // P
        a = stk.reshape(n_ci, P, N_DMA_GROUPS, GT)
        return np.ascontiguousarray(
            a.transpose(2, 1, 0, 3).reshape(N_DMA_GROUPS * P, n_ci * GT))

    in_maps = []
    for i in range(N_CORES):
        xs = np.ascontiguousarray(x[i * T_CORE:(i + 1) * T_CORE].T)
        x_terms = cfg["make_x"](xs)  # each [4096, 2048]
        xstk = np.concatenate(
            [xt.astype(mybir.dt.np(cfg["x_dt"])) for xt in x_terms], axis=0)
        m = {
            "xstk": group_major(xstk),
            "wstk": wstk,
            "biasb": biasb,
        }
        if f8cross:
            xh = xs.astype(np.float16).astype(np.float32)
            xl = xs - xh
            if cfg.get("dev_cast", False):
                x8 = (xl * CROSS_SCALE).astype(f8np)
            else:
                x8 = np.concatenate([
                    (xl * CROSS_SCALE).astype(f8np),
                    xh.astype(f8np),
                ], axis=0)
            m["x8stk"] = group_major(x8)
            m["w8stk"] = w8stk
        in_maps.append(m)
    return in_maps


def kernel(hidden_states, weight, e_score_correction_bias):
    runner = _get_runner()
    results = runner.run(
        make_in_maps(hidden_states, weight, e_score_correction_bias)
    )
    topk_idx = np.concatenate(
        [r["idx_out"].astype(np.int32) for r in results], axis=0
    )
    topk_weight = np.concatenate([r["w_out"] for r in results], axis=0)
    return topk_idx, topk_weight


# revision 47
# speedup vs baseline: 270.9532x; 1.4100x over previous
"""MiMo V2 MoE gate (sigmoid routing, grouped top-k) on 8 Trainium2 cores.

Contract: kernel(**inputs) takes the FULL unsharded inputs
(hidden_states [4,4096,4096] f32, weight [256,4096] f32,
e_score_correction_bias [256] f32) and returns (topk_idx int32 [16384,8],
topk_weight f32 [16384,8]) matching the MiMo V2 MoE gate reference
(sigmoid scores, bias-corrected grouped top-4-of-8 groups by top-2 sums,
top-8 experts, sum-normalized weights scaled by 2.5).

Strategy (data-parallel over tokens):
  - 16384 tokens sharded 2048/core across 8 NeuronCores.
  - Gate GEMM in 1.5 pass-equivalents ("f8crossd"): x and w split into
    fp16 hi + lo; the hi*hi pass runs in fp16 with x pre-scaled by 2^9
    (exact power-of-2), and BOTH cross terms (xl*wh + xh*wl) run as ONE
    K-stacked fp8e5m2 DoubleRow pass at 0.5 cycles/row, pre-scaled by
    2^9 so all passes share a single PSUM accumulation; the 2^-9
    descale folds into the sigmoid's scale operand (zero extra ops).
    The xh8 fp8 operand is derived on-device (ScalarE cast, pipelined
    one DMA group ahead), saving 8 MiB/core of DMA. Routing-rank error
    vs the fp32 reference: 23/131072 flipped indices (rel-err 9.5e-3,
    inside the 2e-2 gate; HW matches the host fp8 simulation exactly).
    Dropping to a single fp16/fp32r pass mis-ranks too many near-ties
    (225/152 flips, rel-err > 2e-2) and walrus forbids mixing 32-bit
    and 16-bit matmul operands, so this is the cheapest passing GEMM.
  - x is staged group-major ([group, partition, chunk, token]) so each
    partition's DMA-group slice is one contiguous run (minimal
    descriptor count, linear HBM walk).
  - Per core: PSUM-accumulated GEMM per 128-token tile, sigmoid on
    ScalarE, grouped top-k on VectorE (DVE sort8 primitives), weights
    recovered via a masked re-sort + 8x8 index match done BATCHED across
    all 16 tiles at the end.
  - CoreSim: single-exec 121 us, steady-state marginal 82 us (split3
    alternative: 164 us marginal); HW reps-line measurements bound the
    real exec to sim-level (dispatch wall is flat in body replication
    to +-250 us resolution).
"""

import sys

if "/opt/trn_rl_repo" not in sys.path:
    sys.path.insert(0, "/opt/trn_rl_repo")

import numpy as np

import concourse.bass as bass
import concourse.mybir as mybir
import concourse.tile as tile
from concourse.tile_rust import add_dep_helper, annotate_deps

P = 128
H = 4096
E = 256
N_CORES = 8
T_FULL = 16384
T_CORE = T_FULL // N_CORES  # 2048
KC = H // P                 # 32 contraction chunks per GEMM term
GT = 128                    # tokens per DMA group (group-major staging
                            # keeps per-partition runs contiguous; small
                            # groups minimize startup serialization:
                            # sim single-exec 121us vs 154us at GT=512)
N_DMA_GROUPS = T_CORE // GT  # 4
TILES_PER_GROUP = GT // P   # 4
TOK_TILES = T_CORE // P     # 16 token tiles per core
N_GROUP = 8
EG = E // N_GROUP           # 32 experts per group
TOPK_GROUP = 4
TOP_K = 8
ROUTED_SCALING = 2.5
NEG = -1e30

F32 = mybir.dt.float32
F32R = mybir.dt.float32r
F16 = mybir.dt.float16
F8E5 = mybir.dt.float8e5
U32 = mybir.dt.uint32
AF = mybir.ActivationFunctionType
OP = mybir.AluOpType

CROSS_SCALE = 512.0  # 2^9: exact power-of-2 pre-scale for the fp8 cross
                     # pass so it shares one PSUM with the scaled hi pass


def _split16(a):
    """a (f32) -> (hi, lo) fp16 with hi + lo ~= a to ~2^-22 relative."""
    hi = a.astype(np.float16)
    lo = (a - hi.astype(np.float32)).astype(np.float16)
    return hi, lo


# A scheme is (name, x_dt, w_dt, n_x, n_w, terms, make_x, make_w) where
# terms is a list of (x_term_idx, w_term_idx) GEMM passes accumulated in
# PSUM, and make_x/make_w map the f32 host array -> list of term arrays.
SCHEMES = {
    # exact fp32 4-pass matmul (slow, bit-accurate baseline)
    "f32": dict(
        x_dt=F32, w_dt=F32, n_x=1, n_w=1, terms=((0, 0),),
        make_x=lambda x: [x.astype(np.float32)],
        make_w=lambda w: [w.astype(np.float32)],
    ),
    # x split hi+lo fp16 (stationary), w exact fp32 bits streamed as
    # float32r moving operand: 2 full-speed passes
    "xsplit_wr": dict(
        x_dt=F16, w_dt=F32R, n_x=2, n_w=1, terms=((0, 0), (1, 0)),
        make_x=lambda x: list(_split16(x)),
        make_w=lambda w: [w.astype(np.float32)],
    ),
    # x split hi+lo fp16, w split hi+lo fp16: 3 full-speed passes
    "split3": dict(
        x_dt=F16, w_dt=F16, n_x=2, n_w=2, terms=((0, 0), (1, 0), (0, 1)),
        make_x=lambda x: list(_split16(x)),
        make_w=lambda w: list(_split16(w)),
    ),
    # single-pass fp16 probe (fails idx tolerance; diagnostics only)
    "f16": dict(
        x_dt=F16, w_dt=F16, n_x=1, n_w=1, terms=((0, 0),),
        make_x=lambda x: [x.astype(np.float16)],
        make_w=lambda w: [w.astype(np.float16)],
    ),
    # x fp16 single (stationary), w float32r moving probe
    "x16_wr": dict(
        x_dt=F16, w_dt=F32R, n_x=1, n_w=1, terms=((0, 0),),
        make_x=lambda x: [x.astype(np.float16)],
        make_w=lambda w: [w.astype(np.float32)],
    ),
    # hi pass fp16 (x pre-scaled by 2^9) + both cross terms K-stacked in
    # one fp8e5m2 DoubleRow pass (also scaled 2^9); the 2^-9 descale is
    # folded into the sigmoid's scale operand. 1.5 pass-equivalents.
    "f8cross": dict(
        x_dt=F16, w_dt=F16, n_x=1, n_w=1, terms=((0, 0),),
        f8cross=True, sig_scale=1.0 / CROSS_SCALE,
        make_x=lambda x: [
            (x.astype(np.float16) * np.float16(CROSS_SCALE))],
        make_w=lambda w: [w.astype(np.float16)],
    ),
    # f8cross with the xh8 half of the cross operand derived ON DEVICE
    # (ScalarE fp16 -> fp8 cast, pipelined one DMA group ahead), saving
    # 8 MiB/core of DMA.
    "f8crossd": dict(
        x_dt=F16, w_dt=F16, n_x=1, n_w=1, terms=((0, 0),),
        f8cross=True, dev_cast=True, sig_scale=1.0 / CROSS_SCALE,
        make_x=lambda x: [
            (x.astype(np.float16) * np.float16(CROSS_SCALE))],
        make_w=lambda w: [w.astype(np.float16)],
    ),
}

SCHEME = "f8crossd"


def _reserve(nc, eng, X, n, prev=None):
    """Emit n plain nops on X's engine, ordered after `prev` (a
    BassInstruction or None) and before X. They act as spare 1-wait
    carriers for _legalize_waits (every TPB instruction has exactly one
    HW wait slot; Tile can assign several waits to one instruction,
    which walrus then rejects)."""
    last = prev.ins if prev is not None else None
    for _ in range(n):
        nop = eng.nop(nofuse=True)
        if last is not None:
            add_dep_helper(nop.ins, last, sync=False,
                           reason="chain reserve nop after predecessor")
        add_dep_helper(X.ins, nop.ins, sync=False,
                       reason="reserve nop precedes its instruction")
        last = nop.ins


def _legalize_waits(nc, report=None):
    """Every TPB instruction has ONE hardware wait slot; Tile can assign
    several on_wait entries to an instruction, which walrus rejects
    ("Too many sync wait commands"). Fix in two ways, per engine stream
    (scheduled order):
      1. value-floor dedup: drop waits already implied by an earlier wait
         on the same semaphore in this stream (monotonic sems).
      2. excess-wait hoisting: move extra waits onto the nearest earlier
         wait-free instruction, scanning only across instructions with no
         on_update (pure nops) -- crossing an updater could reorder a
         producer chain and deadlock; this rule keeps placements provably
         safe. _reserve() plants such nops next to risky instructions.
    Drains are skipped (they encode multi-sem waits natively)."""
    stop_types = (
        mybir.InstDrain,
        mybir.InstEventSemaphore,
        mybir.InstCall,
    )
    leftover = []
    if True:
        # The kernel CFG is linear (main block -> end block), so per-engine
        # program order is the block-order concatenation. Crossing an
        # unconditional branch just means waiting before the jump.
        streams = {}
        nonmono = set()  # sems that ever decrease (barrier sems): no
                         # floor-dedup and no relocation for their waits
        for blk in nc.m.functions[0].blocks:
            for inst in blk.instructions:
                streams.setdefault(str(inst.engine), []).append(inst)
                si = inst.sync_info
                for u in (si.on_update if si and si.on_update else []):
                    if str(u.update_mode) not in ('sem-inc', 'sem-add-imm'):
                        nonmono.add(u.id)
        for stream in streams.values():
            floor = {}
            for i, X in enumerate(stream):
                si = X.sync_info
                if si is None or not si.on_wait:
                    continue
                mode_ok = lambda w: (str(w.wait_mode) == 'sem-ge-imm'
                                     and w.id not in nonmono)
                waits = []
                for w in si.on_wait:
                    if (mode_ok(w) and w.id in floor
                            and floor[w.id] >= w.wait_value):
                        continue  # already implied earlier in this stream
                    waits.append(w)
                moved = []
                if len(waits) > 1:
                    # only sem-ge waits are relocatable; sem-sub barrier
                    # ops must stay exactly where Tile put them
                    fixed = [w for w in waits if not mode_ok(w)]
                    movable = [w for w in waits if mode_ok(w)]
                    keep = fixed + movable[:max(0, 1 - len(fixed))]
                    maybe_move = movable[max(0, 1 - len(fixed)):]
                    for w in maybe_move:
                        placed = False
                        for k in range(i - 1, -1, -1):
                            C = stream[k]
                            if isinstance(C, stop_types):
                                break
                            csi = C.sync_info
                            if csi and csi.on_update:
                                break  # never cross a semaphore producer
                            cw = list(csi.on_wait) if csi and csi.on_wait else []
                            if cw or isinstance(
                                    C, mybir.InstUnconditionalBranch):
                                continue  # occupied/branch; keep scanning
                                          # (same-sequencer waits commute)
                            C.sync_info = mybir.SyncInfo(on_wait=[w],
                                                         on_update=[])
                            placed = True
                            break
                        if placed:
                            moved.append(w)
                        else:
                            keep.append(w)
                    waits = keep
                for w in list(waits) + moved:
                    if mode_ok(w):
                        floor[w.id] = max(floor.get(w.id, 0), w.wait_value)
                X.sync_info = mybir.SyncInfo(
                    on_wait=waits,
                    on_update=list(si.on_update) if si.on_update else [])
                if len(waits) > 1:
                    leftover.append((X.name, str(X.engine),
                                     type(X).__name__, len(waits)))
    # The PE gate ENGINE_NOPs carry AP operands purely for Tile dep
    # tracking; walrus's engine check rejects a nop with operands, so
    # strip them now (tile.py does the same for InstNoOp instructions).
    for blk in nc.m.functions[0].blocks:
        for inst in blk.instructions:
            if (isinstance(inst, mybir.InstISA) and (inst.ins or inst.outs)
                    and inst.op_name == 'ENGINE_NOP'):
                inst.ins = []
                inst.outs = []

    if report is not None:
        report.extend(leftover)
    elif leftover:
        raise RuntimeError(f"wait legalization failed for: {leftover}")


def build_nc(scheme=SCHEME, reps=1, stage="full"):
    """stage: 'full' | 'nodve' (gemm+sigmoid) | 'gemm' | 'dma' —
    partial builds for HW bottleneck isolation via reps-slope timing."""
    cfg = SCHEMES[scheme]
    x_dt, w_dt = cfg["x_dt"], cfg["w_dt"]
    n_x, n_w, terms = cfg["n_x"], cfg["n_w"], cfg["terms"]
    f8cross = cfg.get("f8cross", False)
    dev_cast = cfg.get("dev_cast", False)
    sig_scale = cfg.get("sig_scale", 1.0)

    nc = bass.Bass()

    # x is staged group-major on the host: [g, p, c, t] so each
    # partition's slice of a DMA group is ONE contiguous run (1-2 DMA
    # descriptors per partition instead of n_x*KC strided 1KB lines).
    xstk = nc.dram_tensor("xstk", [N_DMA_GROUPS * P, n_x * KC * GT], x_dt,
                          kind="ExternalInput")
    wstk = nc.dram_tensor("wstk", [n_w * H, E], w_dt, kind="ExternalInput")
    biasb = nc.dram_tensor("biasb", [P, E], F32, kind="ExternalInput")
    if f8cross:
        n_x8 = 1 if dev_cast else 2
        x8stk = nc.dram_tensor("x8stk",
                               [N_DMA_GROUPS * P, n_x8 * KC * GT], F8E5,
                               kind="ExternalInput")
        w8stk = nc.dram_tensor("w8stk", [2 * H, E], F8E5,
                               kind="ExternalInput")
        x84 = x8stk.ap().rearrange("(g p) (c t) -> g p c t",
                                   g=N_DMA_GROUPS, c=n_x8 * KC)
        w83 = w8stk.ap().rearrange("(c p) e -> p c e", p=P)  # [128, 64, 256]
    idx_out = nc.dram_tensor("idx_out", [T_CORE, TOP_K], U32, kind="ExternalOutput")
    w_out = nc.dram_tensor("w_out", [T_CORE, TOP_K], F32, kind="ExternalOutput")

    x4 = xstk.ap().rearrange("(g p) (c t) -> g p c t",
                             g=N_DMA_GROUPS, c=n_x * KC)
    w3 = wstk.ap().rearrange("(c p) e -> p c e", p=P)   # [128, n_w*32, 256]
    idx3 = idx_out.ap().rearrange("(j p) k -> p j k", p=P)  # [128, 16, 8]
    wo3 = w_out.ap().rearrange("(j p) k -> p j k", p=P)

    with tile.TileContext(nc) as tc:
        with (
            tc.tile_pool(name="const", bufs=1) as cpool,
            tc.tile_pool(name="xin", bufs=2) as xpool,
            tc.tile_pool(name="psum", bufs=8, space="PSUM") as pspool,
            tc.tile_pool(name="work", bufs=2) as wpool,
        ):
            wsb = cpool.tile([P, n_w * KC, E], w_dt)
            nc.sync.dma_start(wsb[:], w3)
            if f8cross:
                w8sb = cpool.tile([P, 2 * KC, E], F8E5)
                nc.sync.dma_start(w8sb[:], w83)
            bsb = cpool.tile([P, E], F32)
            nc.sync.dma_start(bsb[:], biasb.ap())
            negc = cpool.tile([P, E], F32)
            mset = nc.vector.memset(negc[:], NEG)
            # DVE gate: absorb the biasb-DMA wait once, up front, so the
            # first badd doesn't need two hardware wait slots.
            dve_gate = nc.vector.nop(nofuse=True)
            dve_gate.ins.ins = [nc.vector.lower_ap(bsb[0:1, 0:1])]
            dve_gate.ins.outs = []
            annotate_deps(tc.dep_state, dve_gate.ins, tc.shadow_memory,
                          tc._rust_ctx, nc.inst_map)
            # Persistent per-core tiles: no slot reuse, so producers never
            # wait on cross-tile consumers or output DMAs.
            idx_all = cpool.tile([P, TOK_TILES, TOP_K], U32)
            max8_all = cpool.tile([P, TOK_TILES, TOP_K], F32)
            smax8_all = cpool.tile([P, TOK_TILES, TOP_K], F32)
            sidx8_all = cpool.tile([P, TOK_TILES, TOP_K], U32)
            w_all = cpool.tile([P, TOK_TILES, TOP_K], F32)
            if stage != "full":
                # partial builds skip the producers; keep outputs defined
                for t in (idx_all, max8_all, smax8_all, sidx8_all, w_all):
                    nc.vector.memset(t[:], 0)

            prev_sig = None
            prev_mm = None
            prev_dma = None
            prev_dve = mset
            n_groups_total = reps * N_DMA_GROUPS
            group_tiles = {}

            def emit_group_load(gi):
                nonlocal prev_dma, prev_sig
                g = gi % N_DMA_GROUPS
                xg = xpool.tile([P, n_x * KC, GT], x_dt, tag="xg")
                d = nc.sync.dma_start(xg[:], x4[g])
                _reserve(nc, nc.sync, d, 6, prev=prev_dma)
                prev_dma = d
                xg8 = None
                if f8cross:
                    xg8 = xpool.tile([P, 2 * KC, GT], F8E5, tag="xg8")
                    if dev_cast:
                        d8 = nc.sync.dma_start(xg8[:, 0:KC, :], x84[g])
                    else:
                        d8 = nc.sync.dma_start(xg8[:], x84[g])
                    _reserve(nc, nc.sync, d8, 6, prev=prev_dma)
                    prev_dma = d8
                    if dev_cast:
                        # xg holds xh*2^9 fp16; undo the scale during the
                        # fp8 cast so xh8 pairs with wl*2^9.
                        cast = nc.scalar.activation(
                            xg8[:, KC:2 * KC, :], xg[:], AF.Copy,
                            scale=1.0 / CROSS_SCALE)
                        _reserve(nc, nc.scalar, cast, 3, prev=prev_sig)
                        prev_sig = cast
                group_tiles[gi] = (xg, xg8)

            emit_group_load(0)
            for gi in range(n_groups_total):
                if gi + 1 < n_groups_total:
                    emit_group_load(gi + 1)
                xg, xg8 = group_tiles.pop(gi)
                g = gi % N_DMA_GROUPS
                if stage == "dma":
                    continue
                for jj in range(TILES_PER_GROUP):
                    j = g * TILES_PER_GROUP + jj
                    ps = pspool.tile([P, E], F32, tag="ps")
                    # The fused matmul (self-loading LDWEIGHTS) only has
                    # budget for ONE semaphore wait in walrus codegen, but
                    # the tile-leading matmul needs the xg-DMA sem plus the
                    # psum-slot-release sem. Emit a PE NoOp that declares
                    # those data deps (1-elem APs, registered via
                    # annotate_deps) so Tile's per-engine clock absorbs all
                    # waits there; the matmuls then follow wait-free in PE
                    # program order. Tile strips APs from InstNoOp at
                    # lowering, so walrus only sees a plain NOP.
                    gate = nc.tensor.nop(nofuse=True)
                    gate.ins.ins = [nc.tensor.lower_ap(xg[0:1, 0, 0:1])]
                    if f8cross:
                        gate.ins.ins.append(
                            nc.tensor.lower_ap(xg8[0:1, 0, 0:1]))
                    gate.ins.outs = [nc.tensor.lower_ap(ps[0:1, 0:1])]
                    annotate_deps(tc.dep_state, gate.ins, tc.shadow_memory,
                                  tc._rust_ctx, nc.inst_map)
                    _reserve(nc, nc.tensor, gate, 4, prev=prev_mm)
                    n_mm = len(terms) * KC + (KC if f8cross else 0)
                    i_mm = 0
                    for (xi, wi) in terms:
                        for c in range(KC):
                            mm = nc.tensor.matmul(
                                ps[:],
                                lhsT=xg[:, xi * KC + c,
                                        jj * P:(jj + 1) * P],
                                rhs=wsb[:, wi * KC + c, :],
                                start=(i_mm == 0),
                                stop=(i_mm == n_mm - 1),
                            )
                            i_mm += 1
                    if f8cross:
                        for c in range(KC):
                            mm = nc.tensor.matmul(
                                ps[:],
                                lhsT=xg8[:, 2 * c:2 * c + 2,
                                         jj * P:(jj + 1) * P],
                                rhs=w8sb[:, 2 * c:2 * c + 2, :],
                                start=False,
                                stop=(i_mm == n_mm - 1),
                                perf_mode=mybir.MatmulPerfMode.DoubleRow,
                            )
                            i_mm += 1
                    prev_mm = mm
                    if stage == "gemm":
                        continue

                    # ---- scores / biased scores ----
                    scores = wpool.tile([P, E], F32, tag="scores")
                    sig = nc.scalar.activation(scores[:], ps[:], AF.Sigmoid,
                                               scale=sig_scale)
                    _reserve(nc, nc.scalar, sig, 3, prev=prev_sig)
                    prev_sig = sig
                    if stage == "nodve":
                        continue
                    sfc = wpool.tile([P, E], F32, tag="sfc")
                    badd = nc.vector.tensor_add(sfc[:], scores[:], bsb[:])
                    _reserve(nc, nc.vector, badd, 3, prev=prev_dve)
                    sfc3 = sfc[:].rearrange("p (g e) -> p g e", g=N_GROUP)

                    # ---- group scores: sum of top-2 per group of 32 ----
                    g3 = wpool.tile([P, N_GROUP, 8], F32, tag="g3")
                    for gg in range(N_GROUP):
                        nc.vector.max(g3[:, gg, :],
                                      sfc[:, gg * EG:(gg + 1) * EG])
                    gsum = wpool.tile([P, N_GROUP], F32, tag="gsum")
                    nc.vector.tensor_add(gsum[:], g3[:, :, 0], g3[:, :, 1])

                    # ---- pick top-4 groups; additive mask 0 / -BIG ----
                    g8 = wpool.tile([P, 8], F32, tag="g8")
                    nc.vector.max(g8[:], gsum[:])
                    gneg = wpool.tile([P, N_GROUP], F32, tag="gneg")
                    nc.vector.tensor_scalar(
                        gneg[:], gsum[:],
                        g8[:, TOPK_GROUP - 1:TOPK_GROUP], NEG,
                        op0=OP.is_lt, op1=OP.mult,
                    )

                    # ---- masked biased scores; top-8 experts ----
                    tmp = wpool.tile([P, E], F32, tag="tmp")
                    tmp3 = tmp[:].rearrange("p (g e) -> p g e", g=N_GROUP)
                    nc.vector.tensor_tensor(
                        tmp3, sfc3,
                        gneg[:, :, None].to_broadcast([P, N_GROUP, EG]),
                        op=OP.add,
                    )
                    max8 = max8_all[:, j, :]
                    nc.vector.max(max8, tmp[:])
                    nc.vector.max_index(idx_all[:, j, :], max8, tmp[:])

                    # ---- unbiased scores of the selected 8 ----
                    zap = wpool.tile([P, E], F32, tag="zap")
                    nc.vector.match_replace(
                        zap[:], in_to_replace=max8, in_values=tmp[:],
                        imm_value=NEG,
                    )
                    sel = wpool.tile([P, E], U32, tag="sel")
                    nc.vector.tensor_tensor(sel[:], tmp[:], zap[:],
                                            op=OP.not_equal)
                    sm = wpool.tile([P, E], F32, tag="sm")
                    nc.vector.select(sm[:], sel[:], scores[:], negc[:])
                    nc.vector.max(smax8_all[:, j, :], sm[:])
                    prev_dve = nc.vector.max_index(sidx8_all[:, j, :],
                                                   smax8_all[:, j, :], sm[:])

            if stage == "full":
                # ---- batched epilogue over all 16 tiles ----
                # reorder scores to biased-rank order:
                #   w8[j,k] = sum_r smax8[j,r] * (sidx8[j,r] == idx8[j,k])
                idxf = cpool.tile([P, TOK_TILES, TOP_K], F32)
                nc.vector.tensor_copy(idxf[:], idx_all[:])
                sidxf = cpool.tile([P, TOK_TILES, TOP_K], F32)
                nc.vector.tensor_copy(sidxf[:], sidx8_all[:])
                eq = cpool.tile([P, TOK_TILES, TOP_K, TOP_K], F32)
                nc.vector.tensor_tensor(
                    eq[:],
                    idxf[:, :, :, None].to_broadcast(
                        [P, TOK_TILES, TOP_K, TOP_K]),
                    sidxf[:, :, None, :].to_broadcast(
                        [P, TOK_TILES, TOP_K, TOP_K]),
                    op=OP.is_equal,
                )
                wprod = cpool.tile([P, TOK_TILES, TOP_K, TOP_K], F32)
                nc.vector.tensor_tensor(
                    wprod[:], eq[:],
                    smax8_all[:, :, None, :].to_broadcast(
                        [P, TOK_TILES, TOP_K, TOP_K]),
                    op=OP.mult,
                )
                w8 = cpool.tile([P, TOK_TILES, TOP_K], F32)
                nc.vector.reduce_sum(w8[:], wprod[:],
                                     axis=mybir.AxisListType.X)

                # ---- normalize: w = 2.5 * w / (sum(w) + 1e-20) ----
                den = cpool.tile([P, TOK_TILES], F32)
                nc.vector.reduce_sum(den[:], w8[:],
                                     axis=mybir.AxisListType.X)
                nc.vector.tensor_scalar_add(den[:], den[:], 1e-20)
                rden = cpool.tile([P, TOK_TILES], F32)
                nc.vector.reciprocal(rden[:], den[:])
                nc.vector.scalar_tensor_tensor(
                    w_all[:], w8[:], ROUTED_SCALING,
                    rden[:, :, None].to_broadcast([P, TOK_TILES, TOP_K]),
                    op0=OP.mult, op1=OP.mult,
                )

            d1 = nc.sync.dma_start(idx3, idx_all[:])
            _reserve(nc, nc.sync, d1, 2, prev=prev_dma)
            d2 = nc.sync.dma_start(wo3, w_all[:])
            _reserve(nc, nc.sync, d2, 2, prev=d1)
            # Tail carriers: Tile's kernel-tail drain on SP waits on every
            # DMA queue sem (12 waits); give the legalizer enough nops.
            tail = d2.ins
            for _ in range(14):
                nop = nc.sync.nop(nofuse=True)
                add_dep_helper(nop.ins, tail, sync=False,
                               reason="tail drain wait carriers")
                tail = nop.ins

    _legalize_waits(nc)
    return nc


class _Runner:
    """Compile-once SPMD runner (mirrors bass2jax.run_bass_via_pjrt's
    multi-core path, but holds the jitted fn so repeated calls don't
    re-trace/re-jit; inputs can stay resident on device for timing).
    With chain=K, one dispatch runs the NEFF K times back-to-back on
    device (chained through the output buffers), which lets test.py
    amortize away per-dispatch host/tunnel overhead and measure the
    per-execution hardware time as a slope."""

    def __init__(self, nc, chain=1, donate=True):
        import jax
        from jax.experimental.shard_map import shard_map
        from jax.sharding import Mesh, NamedSharding, PartitionSpec

        from concourse import bass2jax

        bass2jax.install_neuronx_cc_hook()
        self._jax = jax
        self.nc = nc
        self.chain = chain

        partition_name = (
            nc.partition_id_tensor.name if nc.partition_id_tensor else None
        )
        in_names, out_names, out_avals, zero_outs = [], [], [], []
        for alloc in nc.m.functions[0].allocations:
            if not isinstance(alloc, mybir.MemoryLocationSet):
                continue
            name = alloc.memorylocations[0].name
            if alloc.kind == "ExternalInput":
                if name != partition_name:
                    in_names.append(name)
            elif alloc.kind == "ExternalOutput":
                shape = tuple(alloc.tensor_shape)
                dtype = mybir.dt.np(alloc.dtype)
                out_names.append(name)
                out_avals.append(jax.core.ShapedArray(shape, dtype))
                zero_outs.append(np.zeros(shape, dtype))
        self.in_names = list(in_names)
        self.out_names = out_names
        self.out_avals = out_avals
        self.zero_outs = zero_outs
        n_params = len(in_names)
        self.n_params = n_params

        all_names = in_names + out_names
        if partition_name is not None:
            all_names.append(partition_name)

        def _body(*args):
            ins = list(args[:n_params])
            outs = list(args[n_params:])
            for _ in range(chain):
                operands = ins + list(outs)
                if partition_name is not None:
                    operands.append(bass2jax.partition_id_tensor())
                outs = bass2jax._bass_exec_p.bind(
                    *operands,
                    out_avals=tuple(out_avals),
                    in_names=tuple(all_names),
                    out_names=tuple(out_names),
                    lowering_input_output_aliases=(),
                    sim_require_finite=True,
                    sim_require_nnan=True,
                    nc=nc,
                )
            return tuple(outs)

        devices = jax.devices()[:N_CORES]
        assert len(devices) == N_CORES
        self.mesh = Mesh(np.asarray(devices), ("core",))
        n_outs = len(out_names)
        in_specs = (PartitionSpec("core"),) * (n_params + n_outs)
        out_specs = (PartitionSpec("core"),) * n_outs
        donate_nums = (
            tuple(range(n_params, n_params + n_outs)) if donate else ()
        )
        self._fn = jax.jit(
            shard_map(
                _body, mesh=self.mesh, in_specs=in_specs, out_specs=out_specs,
                check_rep=False,
            ),
            donate_argnums=donate_nums,
            keep_unused=True,
        )
        self._sharding = NamedSharding(self.mesh, PartitionSpec("core"))

    def put_inputs(self, in_maps):
        """Concat per-core inputs on axis 0 and move to device once."""
        concat = [
            np.concatenate([np.asarray(m[name]) for m in in_maps], axis=0)
            for name in self.in_names
        ]
        return [self._jax.device_put(a, self._sharding) for a in concat]

    def _zeros(self):
        return [
            np.zeros((N_CORES * z.shape[0], *z.shape[1:]), z.dtype)
            for z in self.zero_outs
        ]

    _zeros_host = _zeros

    def execute(self, dev_inputs):
        outs = self._fn(*dev_inputs, *self._zeros())
        self._jax.block_until_ready(outs)
        return outs

    def run(self, in_maps):
        dev_inputs = self.put_inputs(in_maps)
        out_arrs = self.execute(dev_inputs)
        return [
            {
                name: np.asarray(out_arrs[i]).reshape(
                    N_CORES, *self.out_avals[i].shape
                )[c]
                for i, name in enumerate(self.out_names)
            }
            for c in range(N_CORES)
        ]


_RUNNER_CACHE = {}


def _get_runner(scheme=SCHEME, chain=1):
    key = (scheme, chain)
    if key not in _RUNNER_CACHE:
        _RUNNER_CACHE[key] = _Runner(build_nc(scheme), chain=chain)
    return _RUNNER_CACHE[key]


def _get_runner_nodonate(scheme=SCHEME):
    key = (scheme, "nodonate")
    if key not in _RUNNER_CACHE:
        _RUNNER_CACHE[key] = _Runner(build_nc(scheme), donate=False)
    return _RUNNER_CACHE[key]


def make_in_maps(hidden_states, weight, e_score_correction_bias,
                 scheme=SCHEME):
    cfg = SCHEMES[scheme]
    f8cross = cfg.get("f8cross", False)
    f8np = mybir.dt.np(F8E5)
    x = np.ascontiguousarray(np.asarray(hidden_states), dtype=np.float32)
    x = x.reshape(T_FULL, H)
    w = np.asarray(weight, dtype=np.float32)
    b = np.asarray(e_score_correction_bias, dtype=np.float32)

    w_terms = cfg["make_w"](w)  # each [256, 4096]
    wstk = np.concatenate([wt.T for wt in w_terms], axis=0)
    wstk = np.ascontiguousarray(wstk, dtype=mybir.dt.np(cfg["w_dt"]))
    biasb = np.ascontiguousarray(np.broadcast_to(b, (P, E)))
    if f8cross:
        whT = w.T.astype(np.float16).astype(np.float32)   # [4096, 256]
        wlT = w.T.astype(np.float32) - whT
        w8stk = np.ascontiguousarray(np.concatenate([
            whT.astype(f8np),
            (wlT * CROSS_SCALE).astype(f8np),
        ], axis=0))

    def group_major(stk):
        """[n*H, T] -> [G*P, n*KC*GT]: per (group, partition) contiguous."""
        n_ci = stk.shape[0] // P
        a = stk.reshape(n_ci, P, N_DMA_GROUPS, GT)
        return np.ascontiguousarray(
            a.transpose(2, 1, 0, 3).reshape(N_DMA_GROUPS * P, n_ci * GT))

    in_maps = []
    for i in range(N_CORES):
        xs = np.ascontiguousarray(x[i * T_CORE:(i + 1) * T_CORE].T)
        x_terms = cfg["make_x"](xs)  # each [4096, 2048]
        xstk = np.concatenate(
            [xt.astype(mybir.dt.np(cfg["x_dt"])) for xt in x_terms], axis=0)
        m = {
            "xstk": group_major(xstk),
            "wstk": wstk,
            "biasb": biasb,
        }
        if f8cross:
            xh = xs.astype(np.float16).astype(np.float32)
            xl = xs - xh
            if cfg.get("dev_cast", False):
                x8 = (xl * CROSS_SCALE).astype(f8np)
            else:
                x8 = np.concatenate([
                    (xl * CROSS_SCALE).astype(f8np),
                    xh.astype(f8np),
                ], axis=0)
            m["x8stk"] = group_major(x8)
            m["w8stk"] = w8stk
        in_maps.append(m)
    return in_maps


def kernel(hidden_states, weight, e_score_correction_bias):
    runner = _get_runner()
    results = runner.run(
        make_in_maps(hidden_states, weight, e_score_correction_bias)
    )
    topk_idx = np.concatenate(
        [r["idx_out"].astype(np.int32) for r in results], axis=0
    )
    topk_weight = np.concatenate([r["w_out"] for r in results], axis=0)
    return topk_idx, topk_weight
